# revision 12
# baseline (speedup 1.0000x reference)
"""Distributed multi-head attention kernel for one TRN2 chip (8 NeuronCores).

Problem: x[4, 2048, 1024] -> qkv Linear(1024, 3072, bias=False) -> 16-head
softmax attention -> proj Linear(1024, 1024) + bias.

Sharding: tensor-parallel over heads. Core c owns heads {2c, 2c+1} (128 of the
1024 qkv feature dims). Each core computes Q/K/V for its head pair over the
full sequence, runs attention per (batch, head), then the chip reshards with
one AllToAll per batch so core c ends up with the full 1024 attention features
for its 1/8 slice of tokens. Each core then applies the full W_proj to its
token slice and the host concatenates the 8 token shards.

Compute is bf16 on the TensorEngine (f32 PSUM accumulation), softmax exp on
the ScalarEngine, copies/divides on the VectorEngine.

Layout notes:
 - x is transposed on the host to xT [C, B*N] so SBUF tiles have the
   contraction dim (C) on partitions for the QKV matmuls.
 - Q and K are produced transposed (QT/KT [128 head-dims, tokens]) which is
   exactly the operand layout for S^T = K Q^T. Scores are built transposed
   (ST [k_tok, q_tok]) so that P^T is directly the lhs-side operand of
   O^T = (V^T P)^T ... i.e. PV needs k_tok on partitions, which ST gives.
 - V is produced in natural [token, head-dim] layout with an extra all-ones
   column per head, so the PV matmul also yields the softmax denominator row
   (row 64 of the [65, q] output) for free.
 - No row-max subtraction: scores are ~N(0,1) after scaling so exp is safe.
"""

import os
import sys

import numpy as np

for _p in ("/opt/trn_rl_repo", "/root/.axon_site/_ro/trn_rl_repo"):
    if os.path.isdir(_p) and _p not in sys.path:
        sys.path.append(_p)

import ml_dtypes  # noqa: E402

B, N, C = 4, 2048, 1024
NUM_HEADS = 16
HEAD_DIM = C // NUM_HEADS  # 64
SCALE = HEAD_DIM**-0.5
NCORES = 8
P = 128  # SBUF partitions
QC = 512  # q-chunk (matmul free dim / PSUM bank)

BF16 = ml_dtypes.bfloat16


def build_attention_nc(NB: int = B, NQ: int = N, CH: int = C):
    """Build + compile the SPMD graph. NB batches of NQ tokens, CH channels.

    Every core runs the same graph; per-core behavior differs only through the
    per-core input shards (wq/wk/wv slices) and the AllToAll.
    """
    import concourse.bass as bass
    import concourse.mybir as mybir
    import concourse.tile as tile
    from concourse import bacc

    f32 = mybir.dt.float32
    bf16 = mybir.dt.bfloat16

    n_qc = NQ // QC  # q chunks per batch
    n_kt = NQ // P  # k tiles per batch
    n_cc = CH // P  # contraction chunks
    TPB = NQ // NCORES  # tokens per core per batch (after reshard)
    n_tt = (TPB + P - 1) // P  # proj token tiles
    n_oc = CH // QC  # proj output chunks

    nc = bacc.Bacc("TRN2", target_bir_lowering=False, debug=False,
                   num_devices=NCORES)

    xT = nc.dram_tensor("xT", [CH, NB * NQ], bf16, kind="ExternalInput").ap()
    wq = nc.dram_tensor("wq", [CH, P], bf16, kind="ExternalInput").ap()
    wk = nc.dram_tensor("wk", [CH, P], bf16, kind="ExternalInput").ap()
    wv = nc.dram_tensor("wv", [CH, P], bf16, kind="ExternalInput").ap()
    wp = nc.dram_tensor("wp", [CH, CH], bf16, kind="ExternalInput").ap()
    bp = nc.dram_tensor("bp", [1, CH], f32, kind="ExternalInput").ap()
    out = nc.dram_tensor("out", [NB * TPB, CH], f32, kind="ExternalOutput").ap()

    from contextlib import ExitStack

    with tile.TileContext(nc) as tc, ExitStack() as ctx:
        const = ctx.enter_context(tc.tile_pool(name="const", bufs=1))
        xt_pool = ctx.enter_context(tc.tile_pool(name="xt", bufs=min(n_cc + 4, 12)))
        qkv_pool = ctx.enter_context(tc.tile_pool(name="qkv", bufs=2))
        pt_pool = ctx.enter_context(tc.tile_pool(name="pt", bufs=2))
        ot_pool = ctx.enter_context(tc.tile_pool(name="ot", bufs=2))
        div_pool = ctx.enter_context(tc.tile_pool(name="div", bufs=3))
        at_pool = ctx.enter_context(tc.tile_pool(name="at", bufs=3 * n_cc))
        y_pool = ctx.enter_context(tc.tile_pool(name="y", bufs=3))
        dram = ctx.enter_context(tc.tile_pool(name="dram", bufs=1, space="DRAM"))
        ps_mm = ctx.enter_context(tc.tile_pool(name="ps_mm", bufs=2, space="PSUM"))
        ps_st = ctx.enter_context(tc.tile_pool(name="ps_st", bufs=2, space="PSUM"))
        ps_ot = ctx.enter_context(tc.tile_pool(name="ps_ot", bufs=2, space="PSUM"))

        # --- resident weights ---
        wq_sb = const.tile([P, n_cc, P], bf16, tag="wq")
        wk_sb = const.tile([P, n_cc, P], bf16, tag="wk")
        wv_sb = const.tile([P, n_cc, P], bf16, tag="wv")
        nc.sync.dma_start(wq_sb[:], wq.rearrange("(cc p) m -> p cc m", p=P))
        nc.sync.dma_start(wk_sb[:], wk.rearrange("(cc p) m -> p cc m", p=P))
        nc.sync.dma_start(wv_sb[:], wv.rearrange("(cc p) m -> p cc m", p=P))
        wp_sb = const.tile([P, n_cc, CH], bf16, tag="wp")
        nc.sync.dma_start(wp_sb[:], wp.rearrange("(cc p) m -> p cc m", p=P))
        bias_row = const.tile([1, CH], f32, tag="bias_row")
        nc.sync.dma_start(bias_row[:], bp[:, :])
        bias_sb = const.tile([P, CH], f32, tag="bias")
        nc.gpsimd.partition_broadcast(bias_sb[:], bias_row[:])

        a2a_in = []
        a2a_out = []
        for b in range(NB):
            a2a_in.append(dram.tile([NCORES * P, TPB], bf16, tag=f"a2a_in{b}", name=f"a2a_in{b}"))
            a2a_out.append(dram.tile([NCORES * P, TPB], bf16, tag=f"a2a_out{b}", name=f"a2a_out{b}"))

        def emit_proj_loads(b):
            """at-tile DMAs for batch b's projection (dep: collective b)."""
            ats = []
            for tt in range(n_tt):
                tpb_t = min(P, TPB - tt * P)
                for cc in range(n_cc):
                    at = at_pool.tile([P, P], bf16, tag="at", name="at_tile")
                    nc.sync.dma_start(
                        at[:, :tpb_t],
                        a2a_out[b][cc * P:(cc + 1) * P,
                                   tt * P:tt * P + tpb_t])
                    ats.append(at)
            return ats

        def emit_proj_compute(b, ats):
            """W_proj + bias for this core's TPB tokens of batch b."""
            for tt in range(n_tt):
                tpb_t = min(P, TPB - tt * P)
                for oc in range(n_oc):
                    ocs = slice(oc * QC, (oc + 1) * QC)
                    yps = ps_mm.tile([P, QC], f32, tag="mm", name="yps_t")
                    for cc in range(n_cc):
                        nc.tensor.matmul(yps[:tpb_t, :],
                                         ats[tt * n_cc + cc][:, :tpb_t],
                                         wp_sb[:, cc, ocs],
                                         start=(cc == 0), stop=(cc == n_cc - 1))
                    y_sb = y_pool.tile([P, QC], f32, tag="y", name="y_tile")
                    nc.vector.tensor_add(y_sb[:tpb_t, :], yps[:tpb_t, :],
                                         bias_sb[:tpb_t, ocs])
                    nc.sync.dma_start(
                        out[b * TPB + tt * P:b * TPB + tt * P + tpb_t, ocs],
                        y_sb[:tpb_t, :])

        def emit_xt(b):
            xts = []
            for cc in range(n_cc):
                t = xt_pool.tile([P, NQ], bf16, tag="xt", name="xt_tile")
                nc.sync.dma_start(
                    t[:], xT[cc * P:(cc + 1) * P, b * NQ:(b + 1) * NQ])
                xts.append(t)
            return xts

        def emit_qkv(xts):
            # QT / KT : [128 head-dims, NQ tokens]
            qt_sb = qkv_pool.tile([P, NQ], bf16, tag="qt")
            kt_sb = qkv_pool.tile([P, NQ], bf16, tag="kt")
            for qc in range(n_qc):
                qs = slice(qc * QC, (qc + 1) * QC)
                qps = ps_mm.tile([P, QC], f32, tag="mm")
                for cc in range(n_cc):
                    nc.tensor.matmul(qps[:], wq_sb[:, cc, :], xts[cc][:, qs],
                                     start=(cc == 0), stop=(cc == n_cc - 1))
                nc.vector.tensor_copy(qt_sb[:, qs], qps[:])
                kps = ps_mm.tile([P, QC], f32, tag="mm")
                for cc in range(n_cc):
                    nc.tensor.matmul(kps[:], wk_sb[:, cc, :], xts[cc][:, qs],
                                     start=(cc == 0), stop=(cc == n_cc - 1))
                nc.vector.tensor_copy(kt_sb[:, qs], kps[:])
            # V natural layout + ones column per head: [tok, 2x(64+1)]
            v_sb = qkv_pool.tile([P, n_kt, 130], bf16, tag="v")
            ones_view = v_sb.rearrange("p t (g c) -> p t g c", g=2)[:, :, :, 64:65]
            nc.vector.memset(ones_view, 1.0)
            for tt in range(n_kt):
                ts_ = slice(tt * P, (tt + 1) * P)
                vps = ps_mm.tile([P, P], f32, tag="mm")
                for cc in range(n_cc):
                    nc.tensor.matmul(vps[:], xts[cc][:, ts_], wv_sb[:, cc, :],
                                     start=(cc == 0), stop=(cc == n_cc - 1))
                dst = v_sb.rearrange("p t (g c) -> p t g c", g=2)[:, tt, :, 0:64]
                nc.vector.tensor_copy(dst, vps.rearrange("p (g c) -> p g c", g=2))
            return qt_sb, kt_sb, v_sb

        next_xts = emit_xt(0)
        next_qkv = emit_qkv(next_xts)
        for b in range(NB):
            qt_sb, kt_sb, v_sb = next_qkv
            if b + 1 < NB:
                # prefetch next batch's x^T now — these DMAs have no deps
                next_xts = emit_xt(b + 1)
            if b > 0:
                # at-tiles for proj(b-1): its AllToAll completes early in
                # this batch, and emitting the loads here keeps them ahead
                # of this batch's collective in the sync DMA stream
                prev_ats = emit_proj_loads(b - 1)

            # ---- attention ----
            # Software-pipelined across q-chunks: scores+exp of chunk qc+1
            # are emitted before the PV matmuls of chunk qc so the
            # ScalarEngine (exp) and TensorEngine (PV) overlap instead of
            # ping-ponging.
            ot_sb = ot_pool.tile([P, NQ], bf16, tag="ot")

            def emit_scores(qc):
                qs = slice(qc * QC, (qc + 1) * QC)
                pt_t = pt_pool.tile([P, 2 * n_kt, QC], bf16, tag="pt",
                                    name="pt_tile")
                for kt in range(n_kt):
                    ks = slice(kt * P, (kt + 1) * P)
                    st = ps_st.tile([P, 2, QC], f32, tag="st", name="st_tile")
                    for h in range(2):
                        hs = slice(64 * h, 64 * (h + 1))
                        nc.tensor.matmul(st[:, h, :], kt_sb[hs, ks],
                                         qt_sb[hs, qs])
                    nc.scalar.activation(pt_t[:, 2 * kt:2 * kt + 2, :], st[:],
                                         mybir.ActivationFunctionType.Exp,
                                         scale=SCALE)
                return pt_t

            def emit_pv(qc, pt_t):
                qs = slice(qc * QC, (qc + 1) * QC)
                # P^T @ [V | 1] per head; row 64 of the result is the
                # softmax denominator
                for h in range(2):
                    ops = ps_ot.tile([65, QC], f32, tag="ot", name="ot_ps")
                    for kt in range(n_kt):
                        nc.tensor.matmul(
                            ops[:], v_sb[:, kt, 65 * h:65 * (h + 1)],
                            pt_t[:, 2 * kt + h, :],
                            start=(kt == 0), stop=(kt == n_kt - 1))
                    drow = div_pool.tile([1, QC], f32, tag="drow", name="drow_t")
                    nc.vector.tensor_copy(drow[:], ops[64:65, :])
                    braw = div_pool.tile([64, QC], f32, tag="braw", name="braw_t")
                    nc.gpsimd.partition_broadcast(braw[:], drow[:])
                    rec = div_pool.tile([64, QC], f32, tag="rec", name="rec_t")
                    nc.vector.reciprocal_approx_fast(rec[:], braw[:])
                    nc.vector.tensor_mul(ot_sb[64 * h:64 * (h + 1), qs],
                                         ops[0:64, :], rec[:])

            # scores(qc+1) before pv(qc); next batch's QKV matmuls are
            # emitted before the last PV chunks so the ScalarEngine's exp
            # backlog covers the QKV window on the TensorEngine
            pend = []
            for qc in range(n_qc):
                pend.append((qc, emit_scores(qc)))
                if len(pend) >= 2 and qc < n_qc - 1:
                    emit_pv(*pend.pop(0))
            if b + 1 < NB:
                next_qkv = emit_qkv(next_xts)
            # proj(b-1) in this batch's PV tail: its at-tiles are ready by
            # now (collective b-1 completed mid-batch), so it never sits
            # ahead of independent work in the PE stream waiting on a
            # collective; the next batch's scores fill the AllToAll window
            if b > 0:
                emit_proj_compute(b - 1, prev_ats)
            for item in pend:
                emit_pv(*item)

            # ---- reshard: chunk j of a2a_in goes to core j ----
            # (gpsimd queues: this DMA waits on the whole batch's attention,
            # and on the sync queues it would head-block the next batch's
            # dep-free xt prefetches)
            nc.gpsimd.dma_start(
                a2a_in[b].rearrange("(j p) t -> p j t", p=P),
                ot_sb.rearrange("p (j t) -> p j t", j=NCORES))
            nc.gpsimd.collective_compute(
                "AllToAll", mybir.AluOpType.bypass,
                replica_groups=[list(range(NCORES))],
                ins=[a2a_in[b][:].opt()], outs=[a2a_out[b][:].opt()])

        last_ats = emit_proj_loads(NB - 1)
        emit_proj_compute(NB - 1, last_ats)

    nc.compile()
    return nc


def make_in_maps(x, W_qkv, W_proj, b_proj, NB=B, NQ=N, CH=C):
    """Shard the full inputs into one input map per core."""
    xT = np.ascontiguousarray(
        x.reshape(NB * NQ, CH).T).astype(BF16)
    wp = np.ascontiguousarray(W_proj).astype(BF16)
    bp = np.ascontiguousarray(b_proj[None, :]).astype(np.float32)
    in_maps = []
    for c in range(NCORES):
        cs = slice(P * c, P * (c + 1))
        in_maps.append({
            "xT": xT,
            "wq": np.ascontiguousarray(W_qkv[:, cs]).astype(BF16),
            "wk": np.ascontiguousarray(W_qkv[:, CH:][:, cs]).astype(BF16),
            "wv": np.ascontiguousarray(W_qkv[:, 2 * CH:][:, cs]).astype(BF16),
            "wp": wp,
            "bp": bp,
        })
    return in_maps


def assemble_output(results, NB=B, NQ=N, CH=C):
    """Concatenate the 8 per-core token shards into the full output."""
    TPB = NQ // NCORES
    full = np.empty((NB, NQ, CH), dtype=np.float32)
    for c in range(NCORES):
        y = np.asarray(results[c]["out"], dtype=np.float32)
        for b in range(NB):
            full[b, TPB * c:TPB * (c + 1), :] = y[b * TPB:(b + 1) * TPB]
    return full


_compiled_nc = None


def kernel(x, W_qkv, W_proj, b_proj):
    global _compiled_nc
    x = np.asarray(x, dtype=np.float32)
    W_qkv = np.asarray(W_qkv, dtype=np.float32)
    W_proj = np.asarray(W_proj, dtype=np.float32)
    b_proj = np.asarray(b_proj, dtype=np.float32)

    if _compiled_nc is None:
        _compiled_nc = build_attention_nc()

    from concourse.bass_utils import run_bass_kernel_spmd

    in_maps = make_in_maps(x, W_qkv, W_proj, b_proj)
    res = run_bass_kernel_spmd(_compiled_nc, in_maps,
                               core_ids=list(range(NCORES)))
    return assemble_output(res.results)


# revision 13
# speedup vs baseline: 1.0281x; 1.0281x over previous
"""Distributed multi-head attention kernel for one TRN2 chip (8 NeuronCores).

Problem: x[4, 2048, 1024] -> qkv Linear(1024, 3072, bias=False) -> 16-head
softmax attention -> proj Linear(1024, 1024) + bias.

Sharding: tensor-parallel over heads. Core c owns heads {2c, 2c+1} (128 of the
1024 qkv feature dims). Each core computes Q/K/V for its head pair over the
full sequence, runs attention per (batch, head), then the chip reshards with
one AllToAll per batch so core c ends up with the full 1024 attention features
for its 1/8 slice of tokens. Each core then applies the full W_proj to its
token slice and the host concatenates the 8 token shards.

Compute is bf16 on the TensorEngine (f32 PSUM accumulation), softmax exp on
the ScalarEngine, copies/divides on the VectorEngine.

Layout notes:
 - x is transposed on the host to xT [C, B*N] so SBUF tiles have the
   contraction dim (C) on partitions for the QKV matmuls.
 - Q and K are produced transposed (QT/KT [128 head-dims, tokens]) which is
   exactly the operand layout for S^T = K Q^T. Scores are built transposed
   (ST [k_tok, q_tok]) so that P^T is directly the lhs-side operand of
   O^T = (V^T P)^T ... i.e. PV needs k_tok on partitions, which ST gives.
 - V is produced in natural [token, head-dim] layout with an extra all-ones
   column per head, so the PV matmul also yields the softmax denominator row
   (row 64 of the [65, q] output) for free.
 - No row-max subtraction: scores are ~N(0,1) after scaling so exp is safe.
"""

import os
import sys

import numpy as np

for _p in ("/opt/trn_rl_repo", "/root/.axon_site/_ro/trn_rl_repo"):
    if os.path.isdir(_p) and _p not in sys.path:
        sys.path.append(_p)

import ml_dtypes  # noqa: E402

B, N, C = 4, 2048, 1024
NUM_HEADS = 16
HEAD_DIM = C // NUM_HEADS  # 64
SCALE = HEAD_DIM**-0.5
NCORES = 8
P = 128  # SBUF partitions
QC = 512  # q-chunk (matmul free dim / PSUM bank)

BF16 = ml_dtypes.bfloat16


def build_attention_nc(NB: int = B, NQ: int = N, CH: int = C):
    """Build + compile the SPMD graph. NB batches of NQ tokens, CH channels.

    Every core runs the same graph; per-core behavior differs only through the
    per-core input shards (wq/wk/wv slices) and the AllToAll.
    """
    import concourse.bass as bass
    import concourse.mybir as mybir
    import concourse.tile as tile
    from concourse import bacc

    f32 = mybir.dt.float32
    bf16 = mybir.dt.bfloat16

    n_qc = NQ // QC  # q chunks per batch
    n_kt = NQ // P  # k tiles per batch
    n_cc = CH // P  # contraction chunks
    TPB = NQ // NCORES  # tokens per core per batch (after reshard)
    n_tt = (TPB + P - 1) // P  # proj token tiles
    n_oc = CH // QC  # proj output chunks

    nc = bacc.Bacc("TRN2", target_bir_lowering=False, debug=False,
                   num_devices=NCORES)

    xT = nc.dram_tensor("xT", [CH, NB * NQ], bf16, kind="ExternalInput").ap()
    wq = nc.dram_tensor("wq", [CH, P], bf16, kind="ExternalInput").ap()
    wk = nc.dram_tensor("wk", [CH, P], bf16, kind="ExternalInput").ap()
    wv = nc.dram_tensor("wv", [CH, P], bf16, kind="ExternalInput").ap()
    wp = nc.dram_tensor("wp", [CH, CH], bf16, kind="ExternalInput").ap()
    bp = nc.dram_tensor("bp", [1, CH], f32, kind="ExternalInput").ap()
    out = nc.dram_tensor("out", [NB * TPB, CH], f32, kind="ExternalOutput").ap()

    from contextlib import ExitStack

    with tile.TileContext(nc) as tc, ExitStack() as ctx:
        const = ctx.enter_context(tc.tile_pool(name="const", bufs=1))
        xt_pool = ctx.enter_context(tc.tile_pool(name="xt", bufs=min(n_cc + 4, 12)))
        qkv_pool = ctx.enter_context(tc.tile_pool(name="qkv", bufs=2))
        pt_pool = ctx.enter_context(tc.tile_pool(name="pt", bufs=2))
        ot_pool = ctx.enter_context(tc.tile_pool(name="ot", bufs=2))
        div_pool = ctx.enter_context(tc.tile_pool(name="div", bufs=3))
        at_pool = ctx.enter_context(tc.tile_pool(name="at", bufs=3 * n_cc))
        y_pool = ctx.enter_context(tc.tile_pool(name="y", bufs=3))
        dram = ctx.enter_context(tc.tile_pool(name="dram", bufs=1, space="DRAM"))
        ps_mm = ctx.enter_context(tc.tile_pool(name="ps_mm", bufs=2, space="PSUM"))
        ps_st = ctx.enter_context(tc.tile_pool(name="ps_st", bufs=2, space="PSUM"))
        ps_ot = ctx.enter_context(tc.tile_pool(name="ps_ot", bufs=2, space="PSUM"))

        # --- resident weights ---
        wq_sb = const.tile([P, n_cc, P], bf16, tag="wq")
        wk_sb = const.tile([P, n_cc, P], bf16, tag="wk")
        wv_sb = const.tile([P, n_cc, P], bf16, tag="wv")
        nc.sync.dma_start(wq_sb[:], wq.rearrange("(cc p) m -> p cc m", p=P))
        nc.sync.dma_start(wk_sb[:], wk.rearrange("(cc p) m -> p cc m", p=P))
        nc.sync.dma_start(wv_sb[:], wv.rearrange("(cc p) m -> p cc m", p=P))
        wp_sb = const.tile([P, n_cc, CH], bf16, tag="wp")
        nc.sync.dma_start(wp_sb[:], wp.rearrange("(cc p) m -> p cc m", p=P))
        bias_row = const.tile([1, CH], f32, tag="bias_row")
        nc.sync.dma_start(bias_row[:], bp[:, :])
        bias_sb = const.tile([P, CH], f32, tag="bias")
        nc.gpsimd.partition_broadcast(bias_sb[:], bias_row[:])

        a2a_in = []
        a2a_out = []
        for b in range(NB):
            a2a_in.append(dram.tile([NCORES * P, TPB], bf16, tag=f"a2a_in{b}", name=f"a2a_in{b}"))
            a2a_out.append(dram.tile([NCORES * P, TPB], bf16, tag=f"a2a_out{b}", name=f"a2a_out{b}"))

        def emit_proj_loads(b):
            """at-tile DMAs for batch b's projection (dep: collective b)."""
            ats = []
            for tt in range(n_tt):
                tpb_t = min(P, TPB - tt * P)
                for cc in range(n_cc):
                    at = at_pool.tile([P, P], bf16, tag="at", name="at_tile")
                    nc.sync.dma_start(
                        at[:, :tpb_t],
                        a2a_out[b][cc * P:(cc + 1) * P,
                                   tt * P:tt * P + tpb_t])
                    ats.append(at)
            return ats

        def emit_proj_compute(b, ats):
            """W_proj + bias for this core's TPB tokens of batch b."""
            for tt in range(n_tt):
                tpb_t = min(P, TPB - tt * P)
                for oc in range(n_oc):
                    ocs = slice(oc * QC, (oc + 1) * QC)
                    yps = ps_mm.tile([P, QC], f32, tag="mm", name="yps_t")
                    for cc in range(n_cc):
                        nc.tensor.matmul(yps[:tpb_t, :],
                                         ats[tt * n_cc + cc][:, :tpb_t],
                                         wp_sb[:, cc, ocs],
                                         start=(cc == 0), stop=(cc == n_cc - 1))
                    y_sb = y_pool.tile([P, QC], f32, tag="y", name="y_tile")
                    nc.vector.tensor_add(y_sb[:tpb_t, :], yps[:tpb_t, :],
                                         bias_sb[:tpb_t, ocs])
                    nc.sync.dma_start(
                        out[b * TPB + tt * P:b * TPB + tt * P + tpb_t, ocs],
                        y_sb[:tpb_t, :])

        def emit_xt(b):
            xts = []
            for cc in range(n_cc):
                t = xt_pool.tile([P, NQ], bf16, tag="xt", name="xt_tile")
                nc.sync.dma_start(
                    t[:], xT[cc * P:(cc + 1) * P, b * NQ:(b + 1) * NQ])
                xts.append(t)
            return xts

        def emit_qkv(xts):
            # QT / KT : [128 head-dims, NQ tokens]
            qt_sb = qkv_pool.tile([P, NQ], bf16, tag="qt")
            kt_sb = qkv_pool.tile([P, NQ], bf16, tag="kt")
            # q-chunks in pairs sharing one weight load per c-chunk (the
            # stationary operand reload otherwise serializes with each MM)
            for qc in range(0, n_qc, 2):
                for w_sb, dst in ((wq_sb, qt_sb), (wk_sb, kt_sb)):
                    qsa = slice(qc * QC, (qc + 1) * QC)
                    qsb = slice((qc + 1) * QC, (qc + 2) * QC)
                    psa = ps_mm.tile([P, QC], f32, tag="mm", name="ps_a")
                    psb = ps_mm.tile([P, QC], f32, tag="mm", name="ps_b")
                    for cc in range(n_cc):
                        nc.tensor.matmul(psa[:], w_sb[:, cc, :],
                                         xts[cc][:, qsa],
                                         start=(cc == 0), stop=(cc == n_cc - 1))
                        nc.tensor.matmul(psb[:], w_sb[:, cc, :],
                                         xts[cc][:, qsb],
                                         start=(cc == 0), stop=(cc == n_cc - 1))
                    nc.vector.tensor_copy(dst[:, qsa], psa[:])
                    nc.vector.tensor_copy(dst[:, qsb], psb[:])
            # V natural layout + ones column per head: [tok, 2x(64+1)]
            v_sb = qkv_pool.tile([P, n_kt, 130], bf16, tag="v")
            ones_view = v_sb.rearrange("p t (g c) -> p t g c", g=2)[:, :, :, 64:65]
            nc.vector.memset(ones_view, 1.0)
            for tt in range(n_kt):
                ts_ = slice(tt * P, (tt + 1) * P)
                vps = ps_mm.tile([P, P], f32, tag="mm")
                for cc in range(n_cc):
                    nc.tensor.matmul(vps[:], xts[cc][:, ts_], wv_sb[:, cc, :],
                                     start=(cc == 0), stop=(cc == n_cc - 1))
                dst = v_sb.rearrange("p t (g c) -> p t g c", g=2)[:, tt, :, 0:64]
                nc.vector.tensor_copy(dst, vps.rearrange("p (g c) -> p g c", g=2))
            return qt_sb, kt_sb, v_sb

        next_xts = emit_xt(0)
        next_qkv = emit_qkv(next_xts)
        for b in range(NB):
            qt_sb, kt_sb, v_sb = next_qkv
            if b + 1 < NB:
                # prefetch next batch's x^T now — these DMAs have no deps
                next_xts = emit_xt(b + 1)
            if b > 0:
                # at-tiles for proj(b-1): its AllToAll completes early in
                # this batch, and emitting the loads here keeps them ahead
                # of this batch's collective in the sync DMA stream
                prev_ats = emit_proj_loads(b - 1)

            # ---- attention ----
            # Software-pipelined across q-chunks: scores+exp of chunk qc+1
            # are emitted before the PV matmuls of chunk qc so the
            # ScalarEngine (exp) and TensorEngine (PV) overlap instead of
            # ping-ponging.
            ot_sb = ot_pool.tile([P, NQ], bf16, tag="ot")

            def emit_scores(qc):
                qs = slice(qc * QC, (qc + 1) * QC)
                pt_t = pt_pool.tile([P, 2 * n_kt, QC], bf16, tag="pt",
                                    name="pt_tile")
                for kt in range(n_kt):
                    ks = slice(kt * P, (kt + 1) * P)
                    st = ps_st.tile([P, 2, QC], f32, tag="st", name="st_tile")
                    for h in range(2):
                        hs = slice(64 * h, 64 * (h + 1))
                        nc.tensor.matmul(st[:, h, :], kt_sb[hs, ks],
                                         qt_sb[hs, qs])
                    nc.scalar.activation(pt_t[:, 2 * kt:2 * kt + 2, :], st[:],
                                         mybir.ActivationFunctionType.Exp,
                                         scale=SCALE)
                return pt_t

            def emit_pv(qc, pt_t):
                qs = slice(qc * QC, (qc + 1) * QC)
                # P^T @ [V | 1] per head; row 64 of the result is the
                # softmax denominator
                for h in range(2):
                    ops = ps_ot.tile([65, QC], f32, tag="ot", name="ot_ps")
                    for kt in range(n_kt):
                        nc.tensor.matmul(
                            ops[:], v_sb[:, kt, 65 * h:65 * (h + 1)],
                            pt_t[:, 2 * kt + h, :],
                            start=(kt == 0), stop=(kt == n_kt - 1))
                    drow = div_pool.tile([1, QC], f32, tag="drow", name="drow_t")
                    nc.vector.tensor_copy(drow[:], ops[64:65, :])
                    braw = div_pool.tile([64, QC], f32, tag="braw", name="braw_t")
                    nc.gpsimd.partition_broadcast(braw[:], drow[:])
                    rec = div_pool.tile([64, QC], f32, tag="rec", name="rec_t")
                    nc.vector.reciprocal_approx_fast(rec[:], braw[:])
                    nc.vector.tensor_mul(ot_sb[64 * h:64 * (h + 1), qs],
                                         ops[0:64, :], rec[:])

            # scores(qc+1) before pv(qc); next batch's QKV matmuls are
            # emitted before the last PV chunks so the ScalarEngine's exp
            # backlog covers the QKV window on the TensorEngine
            pend = []
            for qc in range(n_qc):
                pend.append((qc, emit_scores(qc)))
                if len(pend) >= 2 and qc < n_qc - 1:
                    emit_pv(*pend.pop(0))
            if b + 1 < NB:
                next_qkv = emit_qkv(next_xts)
            # proj(b-1) in this batch's PV tail: its at-tiles are ready by
            # now (collective b-1 completed mid-batch), so it never sits
            # ahead of independent work in the PE stream waiting on a
            # collective; the next batch's scores fill the AllToAll window
            if b > 0:
                emit_proj_compute(b - 1, prev_ats)
            for item in pend:
                emit_pv(*item)

            # ---- reshard: chunk j of a2a_in goes to core j ----
            # (gpsimd queues: this DMA waits on the whole batch's attention,
            # and on the sync queues it would head-block the next batch's
            # dep-free xt prefetches)
            nc.gpsimd.dma_start(
                a2a_in[b].rearrange("(j p) t -> p j t", p=P),
                ot_sb.rearrange("p (j t) -> p j t", j=NCORES))
            nc.gpsimd.collective_compute(
                "AllToAll", mybir.AluOpType.bypass,
                replica_groups=[list(range(NCORES))],
                ins=[a2a_in[b][:].opt()], outs=[a2a_out[b][:].opt()])

        last_ats = emit_proj_loads(NB - 1)
        emit_proj_compute(NB - 1, last_ats)

    nc.compile()
    return nc


def make_in_maps(x, W_qkv, W_proj, b_proj, NB=B, NQ=N, CH=C):
    """Shard the full inputs into one input map per core."""
    xT = np.ascontiguousarray(
        x.reshape(NB * NQ, CH).T).astype(BF16)
    wp = np.ascontiguousarray(W_proj).astype(BF16)
    bp = np.ascontiguousarray(b_proj[None, :]).astype(np.float32)
    in_maps = []
    for c in range(NCORES):
        cs = slice(P * c, P * (c + 1))
        in_maps.append({
            "xT": xT,
            "wq": np.ascontiguousarray(W_qkv[:, cs]).astype(BF16),
            "wk": np.ascontiguousarray(W_qkv[:, CH:][:, cs]).astype(BF16),
            "wv": np.ascontiguousarray(W_qkv[:, 2 * CH:][:, cs]).astype(BF16),
            "wp": wp,
            "bp": bp,
        })
    return in_maps


def assemble_output(results, NB=B, NQ=N, CH=C):
    """Concatenate the 8 per-core token shards into the full output."""
    TPB = NQ // NCORES
    full = np.empty((NB, NQ, CH), dtype=np.float32)
    for c in range(NCORES):
        y = np.asarray(results[c]["out"], dtype=np.float32)
        for b in range(NB):
            full[b, TPB * c:TPB * (c + 1), :] = y[b * TPB:(b + 1) * TPB]
    return full


_compiled_nc = None


def kernel(x, W_qkv, W_proj, b_proj):
    global _compiled_nc
    x = np.asarray(x, dtype=np.float32)
    W_qkv = np.asarray(W_qkv, dtype=np.float32)
    W_proj = np.asarray(W_proj, dtype=np.float32)
    b_proj = np.asarray(b_proj, dtype=np.float32)

    if _compiled_nc is None:
        _compiled_nc = build_attention_nc()

    from concourse.bass_utils import run_bass_kernel_spmd

    in_maps = make_in_maps(x, W_qkv, W_proj, b_proj)
    res = run_bass_kernel_spmd(_compiled_nc, in_maps,
                               core_ids=list(range(NCORES)))
    return assemble_output(res.results)


# revision 14
# speedup vs baseline: 1.0378x; 1.0094x over previous
"""Distributed multi-head attention kernel for one TRN2 chip (8 NeuronCores).

Problem: x[4, 2048, 1024] -> qkv Linear(1024, 3072, bias=False) -> 16-head
softmax attention -> proj Linear(1024, 1024) + bias.

Sharding: tensor-parallel over heads. Core c owns heads {2c, 2c+1} (128 of the
1024 qkv feature dims). Each core computes Q/K/V for its head pair over the
full sequence, runs attention per (batch, head), then the chip reshards with
one AllToAll per batch so core c ends up with the full 1024 attention features
for its 1/8 slice of tokens. Each core then applies the full W_proj to its
token slice and the host concatenates the 8 token shards.

Compute is bf16 on the TensorEngine (f32 PSUM accumulation), softmax exp on
the ScalarEngine, copies/divides on the VectorEngine.

Layout notes:
 - x is transposed on the host to xT [C, B*N] so SBUF tiles have the
   contraction dim (C) on partitions for the QKV matmuls.
 - Q and K are produced transposed (QT/KT [128 head-dims, tokens]) which is
   exactly the operand layout for S^T = K Q^T. Scores are built transposed
   (ST [k_tok, q_tok]) so that P^T is directly the lhs-side operand of
   O^T = (V^T P)^T ... i.e. PV needs k_tok on partitions, which ST gives.
 - V is produced in natural [token, head-dim] layout with an extra all-ones
   column per head, so the PV matmul also yields the softmax denominator row
   (row 64 of the [65, q] output) for free.
 - No row-max subtraction: scores are ~N(0,1) after scaling so exp is safe.
"""

import os
import sys

import numpy as np

for _p in ("/opt/trn_rl_repo", "/root/.axon_site/_ro/trn_rl_repo"):
    if os.path.isdir(_p) and _p not in sys.path:
        sys.path.append(_p)

import ml_dtypes  # noqa: E402

B, N, C = 4, 2048, 1024
NUM_HEADS = 16
HEAD_DIM = C // NUM_HEADS  # 64
SCALE = HEAD_DIM**-0.5
NCORES = 8
P = 128  # SBUF partitions
QC = 512  # q-chunk (matmul free dim / PSUM bank)

BF16 = ml_dtypes.bfloat16


def build_attention_nc(NB: int = B, NQ: int = N, CH: int = C):
    """Build + compile the SPMD graph. NB batches of NQ tokens, CH channels.

    Every core runs the same graph; per-core behavior differs only through the
    per-core input shards (wq/wk/wv slices) and the AllToAll.
    """
    import concourse.bass as bass
    import concourse.mybir as mybir
    import concourse.tile as tile
    from concourse import bacc

    f32 = mybir.dt.float32
    bf16 = mybir.dt.bfloat16

    n_qc = NQ // QC  # q chunks per batch
    n_kt = NQ // P  # k tiles per batch
    n_cc = CH // P  # contraction chunks
    TPB = NQ // NCORES  # tokens per core per batch (after reshard)
    n_tt = (TPB + P - 1) // P  # proj token tiles
    n_oc = CH // QC  # proj output chunks

    nc = bacc.Bacc("TRN2", target_bir_lowering=False, debug=False,
                   num_devices=NCORES)

    xT = nc.dram_tensor("xT", [CH, NB * NQ], bf16, kind="ExternalInput").ap()
    wq = nc.dram_tensor("wq", [CH, P], bf16, kind="ExternalInput").ap()
    wk = nc.dram_tensor("wk", [CH, P], bf16, kind="ExternalInput").ap()
    wv = nc.dram_tensor("wv", [CH, P], bf16, kind="ExternalInput").ap()
    wp = nc.dram_tensor("wp", [CH, CH], bf16, kind="ExternalInput").ap()
    bp = nc.dram_tensor("bp", [1, CH], f32, kind="ExternalInput").ap()
    out = nc.dram_tensor("out", [NB * TPB, CH], f32, kind="ExternalOutput").ap()

    from contextlib import ExitStack

    with tile.TileContext(nc) as tc, ExitStack() as ctx:
        const = ctx.enter_context(tc.tile_pool(name="const", bufs=1))
        xt_pool = ctx.enter_context(tc.tile_pool(name="xt", bufs=min(n_cc + 4, 12)))
        qkv_pool = ctx.enter_context(tc.tile_pool(name="qkv", bufs=2))
        pt_pool = ctx.enter_context(tc.tile_pool(name="pt", bufs=2))
        ot_pool = ctx.enter_context(tc.tile_pool(name="ot", bufs=2))
        div_pool = ctx.enter_context(tc.tile_pool(name="div", bufs=3))
        at_pool = ctx.enter_context(tc.tile_pool(name="at", bufs=3 * n_cc))
        y_pool = ctx.enter_context(tc.tile_pool(name="y", bufs=3))
        dram = ctx.enter_context(tc.tile_pool(name="dram", bufs=1, space="DRAM"))
        ps_mm = ctx.enter_context(tc.tile_pool(name="ps_mm", bufs=4, space="PSUM"))
        ps_st = ctx.enter_context(tc.tile_pool(name="ps_st", bufs=2, space="PSUM"))
        ps_ot = ps_mm

        # --- resident weights ---
        wq_sb = const.tile([P, n_cc, P], bf16, tag="wq")
        wk_sb = const.tile([P, n_cc, P], bf16, tag="wk")
        wv_sb = const.tile([P, n_cc, P], bf16, tag="wv")
        nc.sync.dma_start(wq_sb[:], wq.rearrange("(cc p) m -> p cc m", p=P))
        nc.sync.dma_start(wk_sb[:], wk.rearrange("(cc p) m -> p cc m", p=P))
        nc.sync.dma_start(wv_sb[:], wv.rearrange("(cc p) m -> p cc m", p=P))
        wp_sb = const.tile([P, n_cc, CH], bf16, tag="wp")
        nc.sync.dma_start(wp_sb[:], wp.rearrange("(cc p) m -> p cc m", p=P))
        bias_row = const.tile([1, CH], f32, tag="bias_row")
        nc.sync.dma_start(bias_row[:], bp[:, :])
        bias_sb = const.tile([P, CH], f32, tag="bias")
        nc.gpsimd.partition_broadcast(bias_sb[:], bias_row[:])

        a2a_in = []
        a2a_out = []
        for b in range(NB):
            a2a_in.append(dram.tile([NCORES * P, TPB], bf16, tag=f"a2a_in{b}", name=f"a2a_in{b}"))
            a2a_out.append(dram.tile([NCORES * P, TPB], bf16, tag=f"a2a_out{b}", name=f"a2a_out{b}"))

        def emit_proj_loads(b):
            """at-tile DMAs for batch b's projection (dep: collective b)."""
            ats = []
            for tt in range(n_tt):
                tpb_t = min(P, TPB - tt * P)
                for cc in range(n_cc):
                    at = at_pool.tile([P, P], bf16, tag="at", name="at_tile")
                    nc.sync.dma_start(
                        at[:, :tpb_t],
                        a2a_out[b][cc * P:(cc + 1) * P,
                                   tt * P:tt * P + tpb_t])
                    ats.append(at)
            return ats

        def emit_proj_compute(b, ats):
            """W_proj + bias for this core's TPB tokens of batch b."""
            for tt in range(n_tt):
                tpb_t = min(P, TPB - tt * P)
                for oc in range(n_oc):
                    ocs = slice(oc * QC, (oc + 1) * QC)
                    yps = ps_mm.tile([P, QC], f32, tag="mm", name="yps_t")
                    for cc in range(n_cc):
                        nc.tensor.matmul(yps[:tpb_t, :],
                                         ats[tt * n_cc + cc][:, :tpb_t],
                                         wp_sb[:, cc, ocs],
                                         start=(cc == 0), stop=(cc == n_cc - 1))
                    y_sb = y_pool.tile([P, QC], f32, tag="y", name="y_tile")
                    nc.vector.tensor_add(y_sb[:tpb_t, :], yps[:tpb_t, :],
                                         bias_sb[:tpb_t, ocs])
                    nc.sync.dma_start(
                        out[b * TPB + tt * P:b * TPB + tt * P + tpb_t, ocs],
                        y_sb[:tpb_t, :])

        def emit_xt(b):
            xts = []
            for cc in range(n_cc):
                t = xt_pool.tile([P, NQ], bf16, tag="xt", name="xt_tile")
                nc.sync.dma_start(
                    t[:], xT[cc * P:(cc + 1) * P, b * NQ:(b + 1) * NQ])
                xts.append(t)
            return xts

        def emit_qkv(xts):
            # QT / KT : [128 head-dims, NQ tokens]
            qt_sb = qkv_pool.tile([P, NQ], bf16, tag="qt")
            kt_sb = qkv_pool.tile([P, NQ], bf16, tag="kt")
            # q-chunks in pairs sharing one weight load per c-chunk (the
            # stationary operand reload otherwise serializes with each MM)
            for qc in range(0, n_qc, 2):
                for w_sb, dst in ((wq_sb, qt_sb), (wk_sb, kt_sb)):
                    qsa = slice(qc * QC, (qc + 1) * QC)
                    qsb = slice((qc + 1) * QC, (qc + 2) * QC)
                    psa = ps_mm.tile([P, QC], f32, tag="mm", name="ps_a")
                    psb = ps_mm.tile([P, QC], f32, tag="mm", name="ps_b")
                    for cc in range(n_cc):
                        nc.tensor.matmul(psa[:], w_sb[:, cc, :],
                                         xts[cc][:, qsa],
                                         start=(cc == 0), stop=(cc == n_cc - 1))
                        nc.tensor.matmul(psb[:], w_sb[:, cc, :],
                                         xts[cc][:, qsb],
                                         start=(cc == 0), stop=(cc == n_cc - 1))
                    nc.vector.tensor_copy(dst[:, qsa], psa[:])
                    nc.vector.tensor_copy(dst[:, qsb], psb[:])
            # V natural layout + ones column per head: [tok, 2x(64+1)]
            v_sb = qkv_pool.tile([P, n_kt, 130], bf16, tag="v")
            ones_view = v_sb.rearrange("p t (g c) -> p t g c", g=2)[:, :, :, 64:65]
            nc.vector.memset(ones_view, 1.0)
            for tt in range(n_kt):
                ts_ = slice(tt * P, (tt + 1) * P)
                vps = ps_mm.tile([P, P], f32, tag="mm")
                for cc in range(n_cc):
                    nc.tensor.matmul(vps[:], xts[cc][:, ts_], wv_sb[:, cc, :],
                                     start=(cc == 0), stop=(cc == n_cc - 1))
                dst = v_sb.rearrange("p t (g c) -> p t g c", g=2)[:, tt, :, 0:64]
                nc.vector.tensor_copy(dst, vps.rearrange("p (g c) -> p g c", g=2))
            return qt_sb, kt_sb, v_sb

        next_xts = emit_xt(0)
        next_qkv = emit_qkv(next_xts)
        for b in range(NB):
            qt_sb, kt_sb, v_sb = next_qkv
            if b + 1 < NB:
                # prefetch next batch's x^T now — these DMAs have no deps
                next_xts = emit_xt(b + 1)
            if b > 0:
                # at-tiles for proj(b-1): its AllToAll completes early in
                # this batch, and emitting the loads here keeps them ahead
                # of this batch's collective in the sync DMA stream
                prev_ats = emit_proj_loads(b - 1)

            # ---- attention ----
            # Software-pipelined across q-chunks: scores+exp of chunk qc+1
            # are emitted before the PV matmuls of chunk qc so the
            # ScalarEngine (exp) and TensorEngine (PV) overlap instead of
            # ping-ponging.
            ot_sb = ot_pool.tile([P, NQ], bf16, tag="ot")

            def emit_scores(qc):
                qs = slice(qc * QC, (qc + 1) * QC)
                pt_t = pt_pool.tile([P, 2 * n_kt, QC], bf16, tag="pt",
                                    name="pt_tile")
                for kt in range(n_kt):
                    ks = slice(kt * P, (kt + 1) * P)
                    st = ps_st.tile([P, 2, QC], f32, tag="st", name="st_tile")
                    for h in range(2):
                        hs = slice(64 * h, 64 * (h + 1))
                        nc.tensor.matmul(st[:, h, :], kt_sb[hs, ks],
                                         qt_sb[hs, qs])
                    nc.scalar.activation(pt_t[:, 2 * kt:2 * kt + 2, :], st[:],
                                         mybir.ActivationFunctionType.Exp,
                                         scale=SCALE)
                return pt_t

            def emit_pv(qc, pt_t):
                qs = slice(qc * QC, (qc + 1) * QC)
                # P^T @ [V | 1] per head; row 64 of the result is the
                # softmax denominator
                for h in range(2):
                    ops = ps_ot.tile([128, QC], f32, tag="mm", name="ot_ps")[:65, :]
                    for kt in range(n_kt):
                        nc.tensor.matmul(
                            ops[:], v_sb[:, kt, 65 * h:65 * (h + 1)],
                            pt_t[:, 2 * kt + h, :],
                            start=(kt == 0), stop=(kt == n_kt - 1))
                    drow = div_pool.tile([1, QC], f32, tag="drow", name="drow_t")
                    nc.vector.tensor_copy(drow[:], ops[64:65, :])
                    braw = div_pool.tile([64, QC], f32, tag="braw", name="braw_t")
                    nc.gpsimd.partition_broadcast(braw[:], drow[:])
                    rec = div_pool.tile([64, QC], f32, tag="rec", name="rec_t")
                    nc.vector.reciprocal_approx_fast(rec[:], braw[:])
                    nc.vector.tensor_mul(ot_sb[64 * h:64 * (h + 1), qs],
                                         ops[0:64, :], rec[:])

            # scores(qc+1) before pv(qc); next batch's QKV matmuls are
            # emitted before the last PV chunks so the ScalarEngine's exp
            # backlog covers the QKV window on the TensorEngine
            pend = []
            for qc in range(n_qc):
                pend.append((qc, emit_scores(qc)))
                if len(pend) >= 2 and qc < n_qc - 1:
                    emit_pv(*pend.pop(0))
            if b + 1 < NB:
                next_qkv = emit_qkv(next_xts)
            # proj(b-1) in this batch's PV tail: its at-tiles are ready by
            # now (collective b-1 completed mid-batch), so it never sits
            # ahead of independent work in the PE stream waiting on a
            # collective; the next batch's scores fill the AllToAll window
            if b > 0:
                emit_proj_compute(b - 1, prev_ats)
            for item in pend:
                emit_pv(*item)

            # ---- reshard: chunk j of a2a_in goes to core j ----
            # (gpsimd queues: this DMA waits on the whole batch's attention,
            # and on the sync queues it would head-block the next batch's
            # dep-free xt prefetches)
            nc.gpsimd.dma_start(
                a2a_in[b].rearrange("(j p) t -> p j t", p=P),
                ot_sb.rearrange("p (j t) -> p j t", j=NCORES))
            nc.gpsimd.collective_compute(
                "AllToAll", mybir.AluOpType.bypass,
                replica_groups=[list(range(NCORES))],
                ins=[a2a_in[b][:].opt()], outs=[a2a_out[b][:].opt()])

        last_ats = emit_proj_loads(NB - 1)
        emit_proj_compute(NB - 1, last_ats)

    nc.compile()
    return nc


def make_in_maps(x, W_qkv, W_proj, b_proj, NB=B, NQ=N, CH=C):
    """Shard the full inputs into one input map per core."""
    xT = np.ascontiguousarray(
        x.reshape(NB * NQ, CH).T).astype(BF16)
    wp = np.ascontiguousarray(W_proj).astype(BF16)
    bp = np.ascontiguousarray(b_proj[None, :]).astype(np.float32)
    in_maps = []
    for c in range(NCORES):
        cs = slice(P * c, P * (c + 1))
        in_maps.append({
            "xT": xT,
            "wq": np.ascontiguousarray(W_qkv[:, cs]).astype(BF16),
            "wk": np.ascontiguousarray(W_qkv[:, CH:][:, cs]).astype(BF16),
            "wv": np.ascontiguousarray(W_qkv[:, 2 * CH:][:, cs]).astype(BF16),
            "wp": wp,
            "bp": bp,
        })
    return in_maps


def assemble_output(results, NB=B, NQ=N, CH=C):
    """Concatenate the 8 per-core token shards into the full output."""
    TPB = NQ // NCORES
    full = np.empty((NB, NQ, CH), dtype=np.float32)
    for c in range(NCORES):
        y = np.asarray(results[c]["out"], dtype=np.float32)
        for b in range(NB):
            full[b, TPB * c:TPB * (c + 1), :] = y[b * TPB:(b + 1) * TPB]
    return full


_compiled_nc = None


def kernel(x, W_qkv, W_proj, b_proj):
    global _compiled_nc
    x = np.asarray(x, dtype=np.float32)
    W_qkv = np.asarray(W_qkv, dtype=np.float32)
    W_proj = np.asarray(W_proj, dtype=np.float32)
    b_proj = np.asarray(b_proj, dtype=np.float32)

    if _compiled_nc is None:
        _compiled_nc = build_attention_nc()

    from concourse.bass_utils import run_bass_kernel_spmd

    in_maps = make_in_maps(x, W_qkv, W_proj, b_proj)
    res = run_bass_kernel_spmd(_compiled_nc, in_maps,
                               core_ids=list(range(NCORES)))
    return assemble_output(res.results)


# revision 15
# speedup vs baseline: 1.0528x; 1.0144x over previous
"""Distributed multi-head attention kernel for one TRN2 chip (8 NeuronCores).

Problem: x[4, 2048, 1024] -> qkv Linear(1024, 3072, bias=False) -> 16-head
softmax attention -> proj Linear(1024, 1024) + bias.

Sharding: tensor-parallel over heads. Core c owns heads {2c, 2c+1} (128 of the
1024 qkv feature dims). Each core computes Q/K/V for its head pair over the
full sequence, runs attention per (batch, head), then the chip reshards with
one AllToAll per batch so core c ends up with the full 1024 attention features
for its 1/8 slice of tokens. Each core then applies the full W_proj to its
token slice and the host concatenates the 8 token shards.

Compute is bf16 on the TensorEngine (f32 PSUM accumulation), softmax exp on
the ScalarEngine, copies/divides on the VectorEngine.

Layout notes:
 - x is transposed on the host to xT [C, B*N] so SBUF tiles have the
   contraction dim (C) on partitions for the QKV matmuls.
 - Q and K are produced transposed (QT/KT [128 head-dims, tokens]) which is
   exactly the operand layout for S^T = K Q^T. Scores are built transposed
   (ST [k_tok, q_tok]) so that P^T is directly the lhs-side operand of
   O^T = (V^T P)^T ... i.e. PV needs k_tok on partitions, which ST gives.
 - V is produced in natural [token, head-dim] layout with an extra all-ones
   column per head, so the PV matmul also yields the softmax denominator row
   (row 64 of the [65, q] output) for free.
 - No row-max subtraction: scores are ~N(0,1) after scaling so exp is safe.
"""

import os
import sys

import numpy as np

for _p in ("/opt/trn_rl_repo", "/root/.axon_site/_ro/trn_rl_repo"):
    if os.path.isdir(_p) and _p not in sys.path:
        sys.path.append(_p)

import ml_dtypes  # noqa: E402

B, N, C = 4, 2048, 1024
NUM_HEADS = 16
HEAD_DIM = C // NUM_HEADS  # 64
SCALE = HEAD_DIM**-0.5
NCORES = 8
P = 128  # SBUF partitions
QC = 512  # q-chunk (matmul free dim / PSUM bank)

BF16 = ml_dtypes.bfloat16


def build_attention_nc(NB: int = B, NQ: int = N, CH: int = C):
    """Build + compile the SPMD graph. NB batches of NQ tokens, CH channels.

    Every core runs the same graph; per-core behavior differs only through the
    per-core input shards (wq/wk/wv slices) and the AllToAll.
    """
    import concourse.bass as bass
    import concourse.mybir as mybir
    import concourse.tile as tile
    from concourse import bacc

    f32 = mybir.dt.float32
    bf16 = mybir.dt.bfloat16

    n_qc = NQ // QC  # q chunks per batch
    n_kt = NQ // P  # k tiles per batch
    n_cc = CH // P  # contraction chunks
    TPB = NQ // NCORES  # tokens per core per batch (after reshard)
    n_tt = (TPB + P - 1) // P  # proj token tiles
    n_oc = CH // QC  # proj output chunks

    nc = bacc.Bacc("TRN2", target_bir_lowering=False, debug=False,
                   num_devices=NCORES)

    xT = nc.dram_tensor("xT", [CH, NB * NQ], bf16, kind="ExternalInput").ap()
    wq = nc.dram_tensor("wq", [CH, P], bf16, kind="ExternalInput").ap()
    wk = nc.dram_tensor("wk", [CH, P], bf16, kind="ExternalInput").ap()
    wv = nc.dram_tensor("wv", [CH, P], bf16, kind="ExternalInput").ap()
    wp = nc.dram_tensor("wp", [CH, CH], bf16, kind="ExternalInput").ap()
    bp = nc.dram_tensor("bp", [1, CH], f32, kind="ExternalInput").ap()
    out = nc.dram_tensor("out", [NB * TPB, CH], f32, kind="ExternalOutput").ap()

    from contextlib import ExitStack

    with tile.TileContext(nc) as tc, ExitStack() as ctx:
        const = ctx.enter_context(tc.tile_pool(name="const", bufs=1))
        xt_pool = ctx.enter_context(tc.tile_pool(name="xt", bufs=min(n_cc + 4, 12)))
        qkv_pool = ctx.enter_context(tc.tile_pool(name="qkv", bufs=2))
        pt_pool = ctx.enter_context(tc.tile_pool(name="pt", bufs=2))
        ot_pool = ctx.enter_context(tc.tile_pool(name="ot", bufs=2))
        div_pool = ctx.enter_context(tc.tile_pool(name="div", bufs=3))
        at_pool = ctx.enter_context(tc.tile_pool(name="at", bufs=3 * n_cc))
        y_pool = ctx.enter_context(tc.tile_pool(name="y", bufs=3))
        dram = ctx.enter_context(tc.tile_pool(name="dram", bufs=1, space="DRAM"))
        ps_mm = ctx.enter_context(tc.tile_pool(name="ps_mm", bufs=4, space="PSUM"))
        ps_st = ctx.enter_context(tc.tile_pool(name="ps_st", bufs=2, space="PSUM"))
        ps_ot = ps_mm

        # --- resident weights ---
        wq_sb = const.tile([P, n_cc, P], bf16, tag="wq")
        wk_sb = const.tile([P, n_cc, P], bf16, tag="wk")
        wv_sb = const.tile([P, n_cc, P], bf16, tag="wv")
        nc.sync.dma_start(wq_sb[:], wq.rearrange("(cc p) m -> p cc m", p=P))
        nc.sync.dma_start(wk_sb[:], wk.rearrange("(cc p) m -> p cc m", p=P))
        nc.sync.dma_start(wv_sb[:], wv.rearrange("(cc p) m -> p cc m", p=P))
        wp_sb = const.tile([P, n_cc, CH], bf16, tag="wp")
        bias_row = const.tile([1, CH], f32, tag="bias_row")
        bias_sb = const.tile([P, CH], f32, tag="bias")

        a2a_in = []
        a2a_out = []
        for b in range(NB):
            a2a_in.append(dram.tile([NCORES * P, TPB], bf16, tag=f"a2a_in{b}", name=f"a2a_in{b}"))
            a2a_out.append(dram.tile([NCORES * P, TPB], bf16, tag=f"a2a_out{b}", name=f"a2a_out{b}"))

        def emit_proj_loads(b):
            """at-tile DMAs for batch b's projection (dep: collective b)."""
            ats = []
            for tt in range(n_tt):
                tpb_t = min(P, TPB - tt * P)
                for cc in range(n_cc):
                    at = at_pool.tile([P, P], bf16, tag="at", name="at_tile")
                    nc.sync.dma_start(
                        at[:, :tpb_t],
                        a2a_out[b][cc * P:(cc + 1) * P,
                                   tt * P:tt * P + tpb_t])
                    ats.append(at)
            return ats

        def emit_proj_compute(b, ats):
            """W_proj + bias for this core's TPB tokens of batch b."""
            for tt in range(n_tt):
                tpb_t = min(P, TPB - tt * P)
                for oc in range(n_oc):
                    ocs = slice(oc * QC, (oc + 1) * QC)
                    yps = ps_mm.tile([P, QC], f32, tag="mm", name="yps_t")
                    for cc in range(n_cc):
                        nc.tensor.matmul(yps[:tpb_t, :],
                                         ats[tt * n_cc + cc][:, :tpb_t],
                                         wp_sb[:, cc, ocs],
                                         start=(cc == 0), stop=(cc == n_cc - 1))
                    y_sb = y_pool.tile([P, QC], f32, tag="y", name="y_tile")
                    nc.vector.tensor_add(y_sb[:tpb_t, :], yps[:tpb_t, :],
                                         bias_sb[:tpb_t, ocs])
                    nc.sync.dma_start(
                        out[b * TPB + tt * P:b * TPB + tt * P + tpb_t, ocs],
                        y_sb[:tpb_t, :])

        def emit_xt(b):
            xts = []
            for cc in range(n_cc):
                t = xt_pool.tile([P, NQ], bf16, tag="xt", name="xt_tile")
                nc.sync.dma_start(
                    t[:], xT[cc * P:(cc + 1) * P, b * NQ:(b + 1) * NQ])
                xts.append(t)
            return xts

        def emit_qkv(xts):
            # QT / KT : [128 head-dims, NQ tokens]
            qt_sb = qkv_pool.tile([P, NQ], bf16, tag="qt")
            kt_sb = qkv_pool.tile([P, NQ], bf16, tag="kt")
            # q-chunks in pairs sharing one weight load per c-chunk (the
            # stationary operand reload otherwise serializes with each MM)
            for qc in range(0, n_qc, 2):
                for w_sb, dst in ((wq_sb, qt_sb), (wk_sb, kt_sb)):
                    qsa = slice(qc * QC, (qc + 1) * QC)
                    qsb = slice((qc + 1) * QC, (qc + 2) * QC)
                    psa = ps_mm.tile([P, QC], f32, tag="mm", name="ps_a")
                    psb = ps_mm.tile([P, QC], f32, tag="mm", name="ps_b")
                    for cc in range(n_cc):
                        nc.tensor.matmul(psa[:], w_sb[:, cc, :],
                                         xts[cc][:, qsa],
                                         start=(cc == 0), stop=(cc == n_cc - 1))
                        nc.tensor.matmul(psb[:], w_sb[:, cc, :],
                                         xts[cc][:, qsb],
                                         start=(cc == 0), stop=(cc == n_cc - 1))
                    nc.vector.tensor_copy(dst[:, qsa], psa[:])
                    nc.vector.tensor_copy(dst[:, qsb], psb[:])
            # V natural layout + ones column per head: [tok, 2x(64+1)]
            v_sb = qkv_pool.tile([P, n_kt, 130], bf16, tag="v")
            ones_view = v_sb.rearrange("p t (g c) -> p t g c", g=2)[:, :, :, 64:65]
            nc.vector.memset(ones_view, 1.0)
            for tt in range(n_kt):
                ts_ = slice(tt * P, (tt + 1) * P)
                vps = ps_mm.tile([P, P], f32, tag="mm")
                for cc in range(n_cc):
                    nc.tensor.matmul(vps[:], xts[cc][:, ts_], wv_sb[:, cc, :],
                                     start=(cc == 0), stop=(cc == n_cc - 1))
                dst = v_sb.rearrange("p t (g c) -> p t g c", g=2)[:, tt, :, 0:64]
                nc.vector.tensor_copy(dst, vps.rearrange("p (g c) -> p g c", g=2))
            return qt_sb, kt_sb, v_sb

        next_xts = emit_xt(0)
        next_qkv = emit_qkv(next_xts)
        # W_proj + bias aren't needed until the first projection — loading
        # them up front would delay the first x^T chunks on the HBM port
        nc.sync.dma_start(wp_sb[:], wp.rearrange("(cc p) m -> p cc m", p=P))
        nc.sync.dma_start(bias_row[:], bp[:, :])
        nc.gpsimd.partition_broadcast(bias_sb[:], bias_row[:])
        for b in range(NB):
            qt_sb, kt_sb, v_sb = next_qkv
            if b + 1 < NB:
                # prefetch next batch's x^T now — these DMAs have no deps
                next_xts = emit_xt(b + 1)
            if b > 0:
                # at-tiles for proj(b-1): its AllToAll completes early in
                # this batch, and emitting the loads here keeps them ahead
                # of this batch's collective in the sync DMA stream
                prev_ats = emit_proj_loads(b - 1)

            # ---- attention ----
            # Software-pipelined across q-chunks: scores+exp of chunk qc+1
            # are emitted before the PV matmuls of chunk qc so the
            # ScalarEngine (exp) and TensorEngine (PV) overlap instead of
            # ping-ponging.
            ot_sb = ot_pool.tile([P, NQ], bf16, tag="ot")

            def emit_scores(qc):
                qs = slice(qc * QC, (qc + 1) * QC)
                pt_t = pt_pool.tile([P, 2 * n_kt, QC], bf16, tag="pt",
                                    name="pt_tile")
                for kt in range(n_kt):
                    ks = slice(kt * P, (kt + 1) * P)
                    st = ps_st.tile([P, 2, QC], f32, tag="st", name="st_tile")
                    for h in range(2):
                        hs = slice(64 * h, 64 * (h + 1))
                        nc.tensor.matmul(st[:, h, :], kt_sb[hs, ks],
                                         qt_sb[hs, qs])
                    nc.scalar.activation(pt_t[:, 2 * kt:2 * kt + 2, :], st[:],
                                         mybir.ActivationFunctionType.Exp,
                                         scale=SCALE)
                return pt_t

            def emit_pv(qc, pt_t):
                qs = slice(qc * QC, (qc + 1) * QC)
                # P^T @ [V | 1] per head; row 64 of the result is the
                # softmax denominator
                for h in range(2):
                    ops = ps_ot.tile([128, QC], f32, tag="mm", name="ot_ps")[:65, :]
                    for kt in range(n_kt):
                        nc.tensor.matmul(
                            ops[:], v_sb[:, kt, 65 * h:65 * (h + 1)],
                            pt_t[:, 2 * kt + h, :],
                            start=(kt == 0), stop=(kt == n_kt - 1))
                    drow = div_pool.tile([1, QC], f32, tag="drow", name="drow_t")
                    nc.vector.tensor_copy(drow[:], ops[64:65, :])
                    braw = div_pool.tile([64, QC], f32, tag="braw", name="braw_t")
                    nc.gpsimd.partition_broadcast(braw[:], drow[:])
                    rec = div_pool.tile([64, QC], f32, tag="rec", name="rec_t")
                    nc.vector.reciprocal_approx_fast(rec[:], braw[:])
                    nc.vector.tensor_mul(ot_sb[64 * h:64 * (h + 1), qs],
                                         ops[0:64, :], rec[:])

            # scores(qc+1) before pv(qc); next batch's QKV matmuls are
            # emitted before the last PV chunks so the ScalarEngine's exp
            # backlog covers the QKV window on the TensorEngine
            pend = []
            for qc in range(n_qc):
                pend.append((qc, emit_scores(qc)))
                if len(pend) >= 2 and qc < n_qc - 1:
                    emit_pv(*pend.pop(0))
            if b + 1 < NB:
                next_qkv = emit_qkv(next_xts)
            # proj(b-1) in this batch's PV tail: its at-tiles are ready by
            # now (collective b-1 completed mid-batch), so it never sits
            # ahead of independent work in the PE stream waiting on a
            # collective; the next batch's scores fill the AllToAll window
            if b > 0:
                emit_proj_compute(b - 1, prev_ats)
            for item in pend:
                emit_pv(*item)

            # ---- reshard: chunk j of a2a_in goes to core j ----
            # (gpsimd queues: this DMA waits on the whole batch's attention,
            # and on the sync queues it would head-block the next batch's
            # dep-free xt prefetches)
            nc.gpsimd.dma_start(
                a2a_in[b].rearrange("(j p) t -> p j t", p=P),
                ot_sb.rearrange("p (j t) -> p j t", j=NCORES))
            nc.gpsimd.collective_compute(
                "AllToAll", mybir.AluOpType.bypass,
                replica_groups=[list(range(NCORES))],
                ins=[a2a_in[b][:].opt()], outs=[a2a_out[b][:].opt()])

        last_ats = emit_proj_loads(NB - 1)
        emit_proj_compute(NB - 1, last_ats)

    nc.compile()
    return nc


def make_in_maps(x, W_qkv, W_proj, b_proj, NB=B, NQ=N, CH=C):
    """Shard the full inputs into one input map per core."""
    xT = np.ascontiguousarray(
        x.reshape(NB * NQ, CH).T).astype(BF16)
    wp = np.ascontiguousarray(W_proj).astype(BF16)
    bp = np.ascontiguousarray(b_proj[None, :]).astype(np.float32)
    in_maps = []
    for c in range(NCORES):
        cs = slice(P * c, P * (c + 1))
        in_maps.append({
            "xT": xT,
            "wq": np.ascontiguousarray(W_qkv[:, cs]).astype(BF16),
            "wk": np.ascontiguousarray(W_qkv[:, CH:][:, cs]).astype(BF16),
            "wv": np.ascontiguousarray(W_qkv[:, 2 * CH:][:, cs]).astype(BF16),
            "wp": wp,
            "bp": bp,
        })
    return in_maps


def assemble_output(results, NB=B, NQ=N, CH=C):
    """Concatenate the 8 per-core token shards into the full output."""
    TPB = NQ // NCORES
    full = np.empty((NB, NQ, CH), dtype=np.float32)
    for c in range(NCORES):
        y = np.asarray(results[c]["out"], dtype=np.float32)
        for b in range(NB):
            full[b, TPB * c:TPB * (c + 1), :] = y[b * TPB:(b + 1) * TPB]
    return full


_compiled_nc = None


def kernel(x, W_qkv, W_proj, b_proj):
    global _compiled_nc
    x = np.asarray(x, dtype=np.float32)
    W_qkv = np.asarray(W_qkv, dtype=np.float32)
    W_proj = np.asarray(W_proj, dtype=np.float32)
    b_proj = np.asarray(b_proj, dtype=np.float32)

    if _compiled_nc is None:
        _compiled_nc = build_attention_nc()

    from concourse.bass_utils import run_bass_kernel_spmd

    in_maps = make_in_maps(x, W_qkv, W_proj, b_proj)
    res = run_bass_kernel_spmd(_compiled_nc, in_maps,
                               core_ids=list(range(NCORES)))
    return assemble_output(res.results)


# revision 19
# speedup vs baseline: 1.0558x; 1.0029x over previous
"""Distributed multi-head attention kernel for one TRN2 chip (8 NeuronCores).

Problem: x[4, 2048, 1024] -> qkv Linear(1024, 3072, bias=False) -> 16-head
softmax attention -> proj Linear(1024, 1024) + bias.

Sharding: tensor-parallel over heads. Core c owns heads {2c, 2c+1} (128 of the
1024 qkv feature dims). Each core computes Q/K/V for its head pair over the
full sequence, runs attention per (batch, head), then the chip reshards with
one AllToAll per batch so core c ends up with the full 1024 attention features
for its 1/8 slice of tokens. Each core then applies the full W_proj to its
token slice and the host concatenates the 8 token shards.

Compute is bf16 on the TensorEngine (f32 PSUM accumulation), softmax exp on
the ScalarEngine, copies/divides on the VectorEngine.

Layout notes:
 - x is transposed on the host to xT [C, B*N] so SBUF tiles have the
   contraction dim (C) on partitions for the QKV matmuls.
 - Q and K are produced transposed (QT/KT [128 head-dims, tokens]) which is
   exactly the operand layout for S^T = K Q^T. Scores are built transposed
   (ST [k_tok, q_tok]) so that P^T is directly the lhs-side operand of
   O^T = (V^T P)^T ... i.e. PV needs k_tok on partitions, which ST gives.
 - V is produced in natural [token, head-dim] layout with an extra all-ones
   column per head, so the PV matmul also yields the softmax denominator row
   (row 64 of the [65, q] output) for free.
 - No row-max subtraction: scores are ~N(0,1) after scaling so exp is safe.
"""

import os
import sys

import numpy as np

for _p in ("/opt/trn_rl_repo", "/root/.axon_site/_ro/trn_rl_repo"):
    if os.path.isdir(_p) and _p not in sys.path:
        sys.path.append(_p)

import ml_dtypes  # noqa: E402

B, N, C = 4, 2048, 1024
NUM_HEADS = 16
HEAD_DIM = C // NUM_HEADS  # 64
SCALE = HEAD_DIM**-0.5
NCORES = 8
P = 128  # SBUF partitions
QC = 512  # q-chunk (matmul free dim / PSUM bank)

BF16 = ml_dtypes.bfloat16


def build_attention_nc(NB: int = B, NQ: int = N, CH: int = C):
    """Build + compile the SPMD graph. NB batches of NQ tokens, CH channels.

    Every core runs the same graph; per-core behavior differs only through the
    per-core input shards (wq/wk/wv slices) and the AllToAll.
    """
    import concourse.bass as bass
    import concourse.mybir as mybir
    import concourse.tile as tile
    from concourse import bacc

    f32 = mybir.dt.float32
    bf16 = mybir.dt.bfloat16

    n_qc = NQ // QC  # q chunks per batch
    n_kt = NQ // P  # k tiles per batch
    n_cc = CH // P  # contraction chunks
    TPB = NQ // NCORES  # tokens per core per batch (after reshard)
    n_tt = (TPB + P - 1) // P  # proj token tiles
    n_oc = CH // QC  # proj output chunks

    nc = bacc.Bacc("TRN2", target_bir_lowering=False, debug=False,
                   num_devices=NCORES)

    xT = nc.dram_tensor("xT", [CH, NB * NQ], bf16, kind="ExternalInput").ap()
    wq = nc.dram_tensor("wq", [CH, P], bf16, kind="ExternalInput").ap()
    wk = nc.dram_tensor("wk", [CH, P], bf16, kind="ExternalInput").ap()
    wv = nc.dram_tensor("wv", [CH, P], bf16, kind="ExternalInput").ap()
    wp = nc.dram_tensor("wp", [CH, CH], bf16, kind="ExternalInput").ap()
    bp = nc.dram_tensor("bp", [1, CH], f32, kind="ExternalInput").ap()
    out = nc.dram_tensor("out", [NB * TPB, CH], f32, kind="ExternalOutput").ap()

    from contextlib import ExitStack

    with tile.TileContext(nc) as tc, ExitStack() as ctx:
        const = ctx.enter_context(tc.tile_pool(name="const", bufs=1))
        xt_pool = ctx.enter_context(tc.tile_pool(name="xt", bufs=min(n_cc + 4, 12)))
        qkv_pool = ctx.enter_context(tc.tile_pool(name="qkv", bufs=2))
        pt_pool = ctx.enter_context(tc.tile_pool(name="pt", bufs=2))
        ot_pool = ctx.enter_context(tc.tile_pool(name="ot", bufs=2))
        div_pool = ctx.enter_context(tc.tile_pool(name="div", bufs=3))
        at_pool = ctx.enter_context(tc.tile_pool(name="at", bufs=3 * n_cc))
        y_pool = ctx.enter_context(tc.tile_pool(name="y", bufs=3))
        dram = ctx.enter_context(tc.tile_pool(name="dram", bufs=1, space="DRAM"))
        ps_mm = ctx.enter_context(tc.tile_pool(name="ps_mm", bufs=4, space="PSUM"))
        ps_st = ctx.enter_context(tc.tile_pool(name="ps_st", bufs=2, space="PSUM"))
        ps_ot = ps_mm

        # --- resident weights ---
        wq_sb = const.tile([P, n_cc, P], bf16, tag="wq")
        wk_sb = const.tile([P, n_cc, P], bf16, tag="wk")
        wv_sb = const.tile([P, n_cc, P], bf16, tag="wv")
        nc.sync.dma_start(wq_sb[:], wq.rearrange("(cc p) m -> p cc m", p=P))
        nc.sync.dma_start(wk_sb[:], wk.rearrange("(cc p) m -> p cc m", p=P))
        nc.sync.dma_start(wv_sb[:], wv.rearrange("(cc p) m -> p cc m", p=P))
        wp_sb = const.tile([P, n_cc, CH], bf16, tag="wp")
        bias_row = const.tile([1, CH], f32, tag="bias_row")
        bias_sb = const.tile([P, CH], f32, tag="bias")

        a2a_in = []
        a2a_out = []
        for b in range(NB):
            a2a_in.append(dram.tile([NCORES * P, TPB], bf16, tag=f"a2a_in{b}", name=f"a2a_in{b}"))
            a2a_out.append(dram.tile([NCORES * P, TPB], bf16, tag=f"a2a_out{b}", name=f"a2a_out{b}"))

        def emit_proj_loads(b):
            """at-tile DMAs for batch b's projection (dep: collective b)."""
            ats = []
            for tt in range(n_tt):
                tpb_t = min(P, TPB - tt * P)
                for cc in range(n_cc):
                    at = at_pool.tile([P, P], bf16, tag="at", name="at_tile")
                    nc.sync.dma_start(
                        at[:, :tpb_t],
                        a2a_out[b][cc * P:(cc + 1) * P,
                                   tt * P:tt * P + tpb_t])
                    ats.append(at)
            return ats

        def emit_proj_compute(b, ats):
            """W_proj + bias for this core's TPB tokens of batch b."""
            for tt in range(n_tt):
                tpb_t = min(P, TPB - tt * P)
                for oc in range(n_oc):
                    ocs = slice(oc * QC, (oc + 1) * QC)
                    yps = ps_mm.tile([P, QC], f32, tag="mm", name="yps_t")
                    for cc in range(n_cc):
                        nc.tensor.matmul(yps[:tpb_t, :],
                                         ats[tt * n_cc + cc][:, :tpb_t],
                                         wp_sb[:, cc, ocs],
                                         start=(cc == 0), stop=(cc == n_cc - 1))
                    y_sb = y_pool.tile([P, QC], f32, tag="y", name="y_tile")
                    nc.vector.tensor_add(y_sb[:tpb_t, :], yps[:tpb_t, :],
                                         bias_sb[:tpb_t, ocs])
                    nc.sync.dma_start(
                        out[b * TPB + tt * P:b * TPB + tt * P + tpb_t, ocs],
                        y_sb[:tpb_t, :])

        def emit_xt(b):
            xts = []
            for cc in range(n_cc):
                t = xt_pool.tile([P, NQ], bf16, tag="xt", name="xt_tile")
                nc.sync.dma_start(
                    t[:], xT[cc * P:(cc + 1) * P, b * NQ:(b + 1) * NQ])
                xts.append(t)
            return xts

        def emit_qkv(xts):
            # QT / KT : [128 head-dims, NQ tokens]
            qt_sb = qkv_pool.tile([P, NQ], bf16, tag="qt")
            kt_sb = qkv_pool.tile([P, NQ], bf16, tag="kt")
            # q-chunks in pairs sharing one weight load per c-chunk (the
            # stationary operand reload otherwise serializes with each MM)
            for qc in range(0, n_qc, 2):
                for w_sb, dst in ((wq_sb, qt_sb), (wk_sb, kt_sb)):
                    qsa = slice(qc * QC, (qc + 1) * QC)
                    qsb = slice((qc + 1) * QC, (qc + 2) * QC)
                    psa = ps_mm.tile([P, QC], f32, tag="mm", name="ps_a")
                    psb = ps_mm.tile([P, QC], f32, tag="mm", name="ps_b")
                    for cc in range(n_cc):
                        nc.tensor.matmul(psa[:], w_sb[:, cc, :],
                                         xts[cc][:, qsa],
                                         start=(cc == 0), stop=(cc == n_cc - 1))
                        nc.tensor.matmul(psb[:], w_sb[:, cc, :],
                                         xts[cc][:, qsb],
                                         start=(cc == 0), stop=(cc == n_cc - 1))
                    nc.vector.tensor_copy(dst[:, qsa], psa[:])
                    nc.vector.tensor_copy(dst[:, qsb], psb[:])
            # V natural layout + ones column per head: [tok, 2x(64+1)]
            v_sb = qkv_pool.tile([P, n_kt, 130], bf16, tag="v")
            ones_view = v_sb.rearrange("p t (g c) -> p t g c", g=2)[:, :, :, 64:65]
            nc.vector.memset(ones_view, 1.0)
            for tt in range(n_kt):
                ts_ = slice(tt * P, (tt + 1) * P)
                vps = ps_mm.tile([P, P], f32, tag="mm")
                for cc in range(n_cc):
                    nc.tensor.matmul(vps[:], xts[cc][:, ts_], wv_sb[:, cc, :],
                                     start=(cc == 0), stop=(cc == n_cc - 1))
                dst = v_sb.rearrange("p t (g c) -> p t g c", g=2)[:, tt, :, 0:64]
                nc.vector.tensor_copy(dst, vps.rearrange("p (g c) -> p g c", g=2))
            return qt_sb, kt_sb, v_sb

        next_xts = emit_xt(0)
        next_qkv = emit_qkv(next_xts)
        # W_proj + bias aren't needed until the first projection — loading
        # them up front would delay the first x^T chunks on the HBM port
        nc.sync.dma_start(wp_sb[:], wp.rearrange("(cc p) m -> p cc m", p=P))
        nc.sync.dma_start(bias_row[:], bp[:, :])
        nc.gpsimd.partition_broadcast(bias_sb[:], bias_row[:])
        for b in range(NB):
            qt_sb, kt_sb, v_sb = next_qkv
            if b + 1 < NB:
                # prefetch next batch's x^T now — these DMAs have no deps
                next_xts = emit_xt(b + 1)
            if b > 0:
                # at-tiles for proj(b-1): its AllToAll completes early in
                # this batch, and emitting the loads here keeps them ahead
                # of this batch's collective in the sync DMA stream
                prev_ats = emit_proj_loads(b - 1)

            # ---- attention ----
            # Software-pipelined across q-chunks: scores+exp of chunk qc+1
            # are emitted before the PV matmuls of chunk qc so the
            # ScalarEngine (exp) and TensorEngine (PV) overlap instead of
            # ping-ponging.
            ot_sb = ot_pool.tile([P, NQ], bf16, tag="ot")

            def emit_scores(qc):
                qs = slice(qc * QC, (qc + 1) * QC)
                pt_t = pt_pool.tile([P, 2 * n_kt, QC], bf16, tag="pt",
                                    name="pt_tile")
                for kt in range(n_kt):
                    ks = slice(kt * P, (kt + 1) * P)
                    st = ps_st.tile([P, 2, QC], f32, tag="st", name="st_tile")
                    for h in range(2):
                        hs = slice(64 * h, 64 * (h + 1))
                        nc.tensor.matmul(st[:, h, :], kt_sb[hs, ks],
                                         qt_sb[hs, qs])
                    nc.scalar.activation(pt_t[:, 2 * kt:2 * kt + 2, :], st[:],
                                         mybir.ActivationFunctionType.Exp,
                                         scale=SCALE)
                return pt_t

            def emit_pv(qc, pt_t):
                qs = slice(qc * QC, (qc + 1) * QC)
                # P^T @ [V | 1] per head; row 64 of the result is the
                # softmax denominator
                for h in range(2):
                    ops = ps_ot.tile([128, QC], f32, tag="mm", name="ot_ps")[:65, :]
                    for kt in range(n_kt):
                        nc.tensor.matmul(
                            ops[:], v_sb[:, kt, 65 * h:65 * (h + 1)],
                            pt_t[:, 2 * kt + h, :],
                            start=(kt == 0), stop=(kt == n_kt - 1))
                    drow = div_pool.tile([1, QC], f32, tag="drow", name="drow_t")
                    nc.vector.tensor_copy(drow[:], ops[64:65, :])
                    braw = div_pool.tile([64, QC], f32, tag="braw", name="braw_t")
                    nc.gpsimd.partition_broadcast(braw[:], drow[:])
                    rec = div_pool.tile([64, QC], f32, tag="rec", name="rec_t")
                    nc.vector.reciprocal_approx_fast(rec[:], braw[:])
                    nc.vector.tensor_mul(ot_sb[64 * h:64 * (h + 1), qs],
                                         ops[0:64, :], rec[:])

            # scores(qc+1) before pv(qc); next batch's QKV matmuls are
            # emitted before the last PV chunks so the ScalarEngine's exp
            # backlog covers the QKV window on the TensorEngine
            pend = []
            for qc in range(n_qc):
                pend.append((qc, emit_scores(qc)))
                if len(pend) >= 2 and qc < n_qc - 1:
                    emit_pv(*pend.pop(0))
            if b + 1 < NB:
                next_qkv = emit_qkv(next_xts)
            # proj(b-1) in this batch's PV tail: its at-tiles are ready by
            # now (collective b-1 completed mid-batch), so it never sits
            # ahead of independent work in the PE stream waiting on a
            # collective; the next batch's scores fill the AllToAll window
            if b > 0:
                emit_proj_compute(b - 1, prev_ats)
            for item in pend:
                emit_pv(*item)

            # ---- reshard: chunk j of a2a_in goes to core j ----
            # (gpsimd queues: this DMA waits on the whole batch's attention,
            # and on the sync queues it would head-block the next batch's
            # dep-free xt prefetches)
            nc.gpsimd.dma_start(
                a2a_in[b].rearrange("(j p) t -> p j t", p=P),
                ot_sb.rearrange("p (j t) -> p j t", j=NCORES))
            nc.gpsimd.collective_compute(
                "AllToAll", mybir.AluOpType.bypass,
                replica_groups=[list(range(NCORES))],
                ins=[a2a_in[b][:].opt()], outs=[a2a_out[b][:].opt()])

        last_ats = emit_proj_loads(NB - 1)
        emit_proj_compute(NB - 1, last_ats)

    nc.compile()
    return nc


def make_in_maps(x, W_qkv, W_proj, b_proj, NB=B, NQ=N, CH=C):
    """Shard the full inputs into one input map per core."""
    xT = np.ascontiguousarray(
        x.reshape(NB * NQ, CH).T).astype(BF16)
    wp = np.ascontiguousarray(W_proj).astype(BF16)
    bp = np.ascontiguousarray(b_proj[None, :]).astype(np.float32)
    in_maps = []
    for c in range(NCORES):
        cs = slice(P * c, P * (c + 1))
        in_maps.append({
            "xT": xT,
            "wq": np.ascontiguousarray(W_qkv[:, cs]).astype(BF16),
            "wk": np.ascontiguousarray(W_qkv[:, CH:][:, cs]).astype(BF16),
            "wv": np.ascontiguousarray(W_qkv[:, 2 * CH:][:, cs]).astype(BF16),
            "wp": wp,
            "bp": bp,
        })
    return in_maps


def assemble_output(results, NB=B, NQ=N, CH=C):
    """Concatenate the 8 per-core token shards into the full output."""
    TPB = NQ // NCORES
    full = np.empty((NB, NQ, CH), dtype=np.float32)
    for c in range(NCORES):
        y = np.asarray(results[c]["out"], dtype=np.float32)
        for b in range(NB):
            full[b, TPB * c:TPB * (c + 1), :] = y[b * TPB:(b + 1) * TPB]
    return full


_compiled_nc = None


def kernel(x, W_qkv, W_proj, b_proj):
    global _compiled_nc
    x = np.asarray(x, dtype=np.float32)
    W_qkv = np.asarray(W_qkv, dtype=np.float32)
    W_proj = np.asarray(W_proj, dtype=np.float32)
    b_proj = np.asarray(b_proj, dtype=np.float32)

    if _compiled_nc is None:
        _compiled_nc = build_attention_nc()

    from concourse.bass_utils import run_bass_kernel_spmd

    in_maps = make_in_maps(x, W_qkv, W_proj, b_proj)
    res = run_bass_kernel_spmd(_compiled_nc, in_maps,
                               core_ids=list(range(NCORES)))
    return assemble_output(res.results)


# revision 20
# speedup vs baseline: 1.0620x; 1.0058x over previous
"""Distributed multi-head attention kernel for one TRN2 chip (8 NeuronCores).

Problem: x[4, 2048, 1024] -> qkv Linear(1024, 3072, bias=False) -> 16-head
softmax attention -> proj Linear(1024, 1024) + bias.

Sharding: tensor-parallel over heads. Core c owns heads {2c, 2c+1} (128 of the
1024 qkv feature dims). Each core computes Q/K/V for its head pair over the
full sequence, runs attention per (batch, head), then the chip reshards with
one AllToAll per batch so core c ends up with the full 1024 attention features
for its 1/8 slice of tokens. Each core then applies the full W_proj to its
token slice and the host concatenates the 8 token shards.

Compute is bf16 on the TensorEngine (f32 PSUM accumulation), softmax exp on
the ScalarEngine, copies/divides on the VectorEngine.

Layout notes:
 - x is transposed on the host to xT [C, B*N] so SBUF tiles have the
   contraction dim (C) on partitions for the QKV matmuls.
 - Q and K are produced transposed (QT/KT [128 head-dims, tokens]) which is
   exactly the operand layout for S^T = K Q^T. Scores are built transposed
   (ST [k_tok, q_tok]) so that P^T is directly the lhs-side operand of
   O^T = (V^T P)^T ... i.e. PV needs k_tok on partitions, which ST gives.
 - V is produced in natural [token, head-dim] layout with an extra all-ones
   column per head, so the PV matmul also yields the softmax denominator row
   (row 64 of the [65, q] output) for free.
 - No row-max subtraction: scores are ~N(0,1) after scaling so exp is safe.
"""

import os
import sys

import numpy as np

for _p in ("/opt/trn_rl_repo", "/root/.axon_site/_ro/trn_rl_repo"):
    if os.path.isdir(_p) and _p not in sys.path:
        sys.path.append(_p)

import ml_dtypes  # noqa: E402

B, N, C = 4, 2048, 1024
NUM_HEADS = 16
HEAD_DIM = C // NUM_HEADS  # 64
SCALE = HEAD_DIM**-0.5
NCORES = 8
P = 128  # SBUF partitions
QC = 512  # q-chunk (matmul free dim / PSUM bank)

BF16 = ml_dtypes.bfloat16


def build_attention_nc(NB: int = B, NQ: int = N, CH: int = C):
    """Build + compile the SPMD graph. NB batches of NQ tokens, CH channels.

    Every core runs the same graph; per-core behavior differs only through the
    per-core input shards (wq/wk/wv slices) and the AllToAll.
    """
    import concourse.bass as bass
    import concourse.mybir as mybir
    import concourse.tile as tile
    from concourse import bacc

    f32 = mybir.dt.float32
    bf16 = mybir.dt.bfloat16

    n_qc = NQ // QC  # q chunks per batch
    n_kt = NQ // P  # k tiles per batch
    n_cc = CH // P  # contraction chunks
    TPB = NQ // NCORES  # tokens per core per batch (after reshard)
    n_tt = (TPB + P - 1) // P  # proj token tiles
    n_oc = CH // QC  # proj output chunks

    nc = bacc.Bacc("TRN2", target_bir_lowering=False, debug=False,
                   num_devices=NCORES)

    xT = nc.dram_tensor("xT", [CH, NB * NQ], bf16, kind="ExternalInput").ap()
    wq = nc.dram_tensor("wq", [CH, P], bf16, kind="ExternalInput").ap()
    wk = nc.dram_tensor("wk", [CH, P], bf16, kind="ExternalInput").ap()
    wv = nc.dram_tensor("wv", [CH, P], bf16, kind="ExternalInput").ap()
    wp = nc.dram_tensor("wp", [CH, CH], bf16, kind="ExternalInput").ap()
    bp = nc.dram_tensor("bp", [1, CH], f32, kind="ExternalInput").ap()
    out = nc.dram_tensor("out", [NB * TPB, CH], f32, kind="ExternalOutput").ap()

    from contextlib import ExitStack

    with tile.TileContext(nc) as tc, ExitStack() as ctx:
        const = ctx.enter_context(tc.tile_pool(name="const", bufs=1))
        xt_pool = ctx.enter_context(tc.tile_pool(name="xt", bufs=min(n_cc + 4, 12)))
        qkv_pool = ctx.enter_context(tc.tile_pool(name="qkv", bufs=2))
        pt_pool = ctx.enter_context(tc.tile_pool(name="pt", bufs=2))
        ot_pool = ctx.enter_context(tc.tile_pool(name="ot", bufs=2))
        div_pool = ctx.enter_context(tc.tile_pool(name="div", bufs=3))
        at_pool = ctx.enter_context(tc.tile_pool(name="at", bufs=3 * n_cc))
        y_pool = ctx.enter_context(tc.tile_pool(name="y", bufs=3))
        dram = ctx.enter_context(tc.tile_pool(name="dram", bufs=1, space="DRAM"))
        ps_mm = ctx.enter_context(tc.tile_pool(name="ps_mm", bufs=4, space="PSUM"))
        ps_st = ctx.enter_context(tc.tile_pool(name="ps_st", bufs=2, space="PSUM"))
        ps_ot = ps_mm

        # --- resident weights ---
        wq_sb = const.tile([P, n_cc, P], bf16, tag="wq")
        wk_sb = const.tile([P, n_cc, P], bf16, tag="wk")
        wv_sb = const.tile([P, n_cc, P], bf16, tag="wv")
        nc.sync.dma_start(wq_sb[:], wq.rearrange("(cc p) m -> p cc m", p=P))
        nc.sync.dma_start(wk_sb[:], wk.rearrange("(cc p) m -> p cc m", p=P))
        nc.sync.dma_start(wv_sb[:], wv.rearrange("(cc p) m -> p cc m", p=P))
        wp_sb = const.tile([P, n_cc, CH], bf16, tag="wp")
        bias_row = const.tile([1, CH], f32, tag="bias_row")
        bias_sb = const.tile([P, CH], f32, tag="bias")

        a2a_in = []
        a2a_out = []
        for b in range(NB):
            a2a_in.append(dram.tile([NCORES * P, TPB], bf16, tag=f"a2a_in{b}", name=f"a2a_in{b}"))
            a2a_out.append(dram.tile([NCORES * P, TPB], bf16, tag=f"a2a_out{b}", name=f"a2a_out{b}"))

        def emit_proj_loads(b):
            """at-tile DMAs for batch b's projection (dep: collective b)."""
            ats = []
            for tt in range(n_tt):
                tpb_t = min(P, TPB - tt * P)
                for cc in range(n_cc):
                    at = at_pool.tile([P, P], bf16, tag="at", name="at_tile")
                    nc.sync.dma_start(
                        at[:, :tpb_t],
                        a2a_out[b][cc * P:(cc + 1) * P,
                                   tt * P:tt * P + tpb_t])
                    ats.append(at)
            return ats

        def emit_proj_compute(b, ats):
            """W_proj + bias for this core's TPB tokens of batch b."""
            for tt in range(n_tt):
                tpb_t = min(P, TPB - tt * P)
                for oc in range(n_oc):
                    ocs = slice(oc * QC, (oc + 1) * QC)
                    yps = ps_mm.tile([P, QC], f32, tag="mm", name="yps_t")
                    for cc in range(n_cc):
                        nc.tensor.matmul(yps[:tpb_t, :],
                                         ats[tt * n_cc + cc][:, :tpb_t],
                                         wp_sb[:, cc, ocs],
                                         start=(cc == 0), stop=(cc == n_cc - 1))
                    y_sb = y_pool.tile([P, QC], f32, tag="y", name="y_tile")
                    nc.vector.tensor_add(y_sb[:tpb_t, :], yps[:tpb_t, :],
                                         bias_sb[:tpb_t, ocs])
                    nc.sync.dma_start(
                        out[b * TPB + tt * P:b * TPB + tt * P + tpb_t, ocs],
                        y_sb[:tpb_t, :])

        def emit_xt(b, split=False):
            xts = []
            halves = 2 if split else 1
            for hv in range(halves):
                for cc in range(n_cc):
                    if hv == 0:
                        t = xt_pool.tile([P, NQ], bf16, tag="xt",
                                         name="xt_tile")
                        xts.append(t)
                    t = xts[cc]
                    hw = NQ // halves
                    nc.sync.dma_start(
                        t[:, hv * hw:(hv + 1) * hw],
                        xT[cc * P:(cc + 1) * P,
                           b * NQ + hv * hw:b * NQ + (hv + 1) * hw])
            return xts

        def emit_qkv(xts):
            # QT / KT : [128 head-dims, NQ tokens]
            qt_sb = qkv_pool.tile([P, NQ], bf16, tag="qt")
            kt_sb = qkv_pool.tile([P, NQ], bf16, tag="kt")
            # q-chunks in pairs sharing one weight load per c-chunk (the
            # stationary operand reload otherwise serializes with each MM)
            for qc in range(0, n_qc, 2):
                for w_sb, dst in ((wq_sb, qt_sb), (wk_sb, kt_sb)):
                    qsa = slice(qc * QC, (qc + 1) * QC)
                    qsb = slice((qc + 1) * QC, (qc + 2) * QC)
                    psa = ps_mm.tile([P, QC], f32, tag="mm", name="ps_a")
                    psb = ps_mm.tile([P, QC], f32, tag="mm", name="ps_b")
                    for cc in range(n_cc):
                        nc.tensor.matmul(psa[:], w_sb[:, cc, :],
                                         xts[cc][:, qsa],
                                         start=(cc == 0), stop=(cc == n_cc - 1))
                        nc.tensor.matmul(psb[:], w_sb[:, cc, :],
                                         xts[cc][:, qsb],
                                         start=(cc == 0), stop=(cc == n_cc - 1))
                    nc.vector.tensor_copy(dst[:, qsa], psa[:])
                    nc.vector.tensor_copy(dst[:, qsb], psb[:])
            # V natural layout + ones column per head: [tok, 2x(64+1)]
            v_sb = qkv_pool.tile([P, n_kt, 130], bf16, tag="v")
            ones_view = v_sb.rearrange("p t (g c) -> p t g c", g=2)[:, :, :, 64:65]
            nc.vector.memset(ones_view, 1.0)
            for tt in range(n_kt):
                ts_ = slice(tt * P, (tt + 1) * P)
                vps = ps_mm.tile([P, P], f32, tag="mm")
                for cc in range(n_cc):
                    nc.tensor.matmul(vps[:], xts[cc][:, ts_], wv_sb[:, cc, :],
                                     start=(cc == 0), stop=(cc == n_cc - 1))
                dst = v_sb.rearrange("p t (g c) -> p t g c", g=2)[:, tt, :, 0:64]
                nc.vector.tensor_copy(dst, vps.rearrange("p (g c) -> p g c", g=2))
            return qt_sb, kt_sb, v_sb

        next_xts = emit_xt(0, split=True)
        next_qkv = emit_qkv(next_xts)
        # W_proj + bias aren't needed until the first projection — loading
        # them up front would delay the first x^T chunks on the HBM port
        nc.sync.dma_start(wp_sb[:], wp.rearrange("(cc p) m -> p cc m", p=P))
        nc.sync.dma_start(bias_row[:], bp[:, :])
        nc.gpsimd.partition_broadcast(bias_sb[:], bias_row[:])
        for b in range(NB):
            qt_sb, kt_sb, v_sb = next_qkv
            if b + 1 < NB:
                # prefetch next batch's x^T now — these DMAs have no deps
                next_xts = emit_xt(b + 1)
            if b > 0:
                # at-tiles for proj(b-1): its AllToAll completes early in
                # this batch, and emitting the loads here keeps them ahead
                # of this batch's collective in the sync DMA stream
                prev_ats = emit_proj_loads(b - 1)

            # ---- attention ----
            # Software-pipelined across q-chunks: scores+exp of chunk qc+1
            # are emitted before the PV matmuls of chunk qc so the
            # ScalarEngine (exp) and TensorEngine (PV) overlap instead of
            # ping-ponging.
            ot_sb = ot_pool.tile([P, NQ], bf16, tag="ot")

            def emit_scores(qc):
                qs = slice(qc * QC, (qc + 1) * QC)
                pt_t = pt_pool.tile([P, 2 * n_kt, QC], bf16, tag="pt",
                                    name="pt_tile")
                for kt in range(n_kt):
                    ks = slice(kt * P, (kt + 1) * P)
                    st = ps_st.tile([P, 2, QC], f32, tag="st", name="st_tile")
                    for h in range(2):
                        hs = slice(64 * h, 64 * (h + 1))
                        nc.tensor.matmul(st[:, h, :], kt_sb[hs, ks],
                                         qt_sb[hs, qs])
                    nc.scalar.activation(pt_t[:, 2 * kt:2 * kt + 2, :], st[:],
                                         mybir.ActivationFunctionType.Exp,
                                         scale=SCALE)
                return pt_t

            def emit_pv(qc, pt_t):
                qs = slice(qc * QC, (qc + 1) * QC)
                # P^T @ [V | 1] per head; row 64 of the result is the
                # softmax denominator
                # the two heads' accumulation chains interleaved MM-by-MM:
                # consecutive matmuls hit different PSUM banks with
                # alternating weight tiles, letting the PE pull the next
                # LDWEIGHTS into the background slot during the current MM
                ops0 = ps_ot.tile([128, QC], f32, tag="mm", name="ot_ps0")[:65, :]
                ops1 = ps_ot.tile([128, QC], f32, tag="mm", name="ot_ps1")[:65, :]
                opss = (ops0, ops1)
                for kt in range(n_kt):
                    for h in range(2):
                        nc.tensor.matmul(
                            opss[h][:], v_sb[:, kt, 65 * h:65 * (h + 1)],
                            pt_t[:, 2 * kt + h, :],
                            start=(kt == 0), stop=(kt == n_kt - 1))
                for h in range(2):
                    ops = opss[h]
                    drow = div_pool.tile([1, QC], f32, tag="drow", name="drow_t")
                    nc.vector.tensor_copy(drow[:], ops[64:65, :])
                    braw = div_pool.tile([64, QC], f32, tag="braw", name="braw_t")
                    nc.gpsimd.partition_broadcast(braw[:], drow[:])
                    rec = div_pool.tile([64, QC], f32, tag="rec", name="rec_t")
                    nc.vector.reciprocal_approx_fast(rec[:], braw[:])
                    nc.vector.tensor_mul(ot_sb[64 * h:64 * (h + 1), qs],
                                         ops[0:64, :], rec[:])

            # scores(qc+1) before pv(qc); next batch's QKV matmuls are
            # emitted before the last PV chunks so the ScalarEngine's exp
            # backlog covers the QKV window on the TensorEngine
            pend = []
            for qc in range(n_qc):
                pend.append((qc, emit_scores(qc)))
                if len(pend) >= 2 and qc < n_qc - 1:
                    emit_pv(*pend.pop(0))
            if b + 1 < NB:
                next_qkv = emit_qkv(next_xts)
            # proj(b-1) in this batch's PV tail: its at-tiles are ready by
            # now (collective b-1 completed mid-batch), so it never sits
            # ahead of independent work in the PE stream waiting on a
            # collective; the next batch's scores fill the AllToAll window
            if b > 0:
                emit_proj_compute(b - 1, prev_ats)
            for item in pend:
                emit_pv(*item)

            # ---- reshard: chunk j of a2a_in goes to core j ----
            # (gpsimd queues: this DMA waits on the whole batch's attention,
            # and on the sync queues it would head-block the next batch's
            # dep-free xt prefetches)
            nc.gpsimd.dma_start(
                a2a_in[b].rearrange("(j p) t -> p j t", p=P),
                ot_sb.rearrange("p (j t) -> p j t", j=NCORES))
            nc.gpsimd.collective_compute(
                "AllToAll", mybir.AluOpType.bypass,
                replica_groups=[list(range(NCORES))],
                ins=[a2a_in[b][:].opt()], outs=[a2a_out[b][:].opt()])

        last_ats = emit_proj_loads(NB - 1)
        emit_proj_compute(NB - 1, last_ats)

    nc.compile()
    return nc


def make_in_maps(x, W_qkv, W_proj, b_proj, NB=B, NQ=N, CH=C):
    """Shard the full inputs into one input map per core."""
    xT = np.ascontiguousarray(
        x.reshape(NB * NQ, CH).T).astype(BF16)
    wp = np.ascontiguousarray(W_proj).astype(BF16)
    bp = np.ascontiguousarray(b_proj[None, :]).astype(np.float32)
    in_maps = []
    for c in range(NCORES):
        cs = slice(P * c, P * (c + 1))
        in_maps.append({
            "xT": xT,
            "wq": np.ascontiguousarray(W_qkv[:, cs]).astype(BF16),
            "wk": np.ascontiguousarray(W_qkv[:, CH:][:, cs]).astype(BF16),
            "wv": np.ascontiguousarray(W_qkv[:, 2 * CH:][:, cs]).astype(BF16),
            "wp": wp,
            "bp": bp,
        })
    return in_maps


def assemble_output(results, NB=B, NQ=N, CH=C):
    """Concatenate the 8 per-core token shards into the full output."""
    TPB = NQ // NCORES
    full = np.empty((NB, NQ, CH), dtype=np.float32)
    for c in range(NCORES):
        y = np.asarray(results[c]["out"], dtype=np.float32)
        for b in range(NB):
            full[b, TPB * c:TPB * (c + 1), :] = y[b * TPB:(b + 1) * TPB]
    return full


_compiled_nc = None


def kernel(x, W_qkv, W_proj, b_proj):
    global _compiled_nc
    x = np.asarray(x, dtype=np.float32)
    W_qkv = np.asarray(W_qkv, dtype=np.float32)
    W_proj = np.asarray(W_proj, dtype=np.float32)
    b_proj = np.asarray(b_proj, dtype=np.float32)

    if _compiled_nc is None:
        _compiled_nc = build_attention_nc()

    from concourse.bass_utils import run_bass_kernel_spmd

    in_maps = make_in_maps(x, W_qkv, W_proj, b_proj)
    res = run_bass_kernel_spmd(_compiled_nc, in_maps,
                               core_ids=list(range(NCORES)))
    return assemble_output(res.results)
